# revision 18
# baseline (speedup 1.0000x reference)
"""Batched GAT kernel for 8 Trainium2 NeuronCores.

Math: out[b,i,:] = softmax_j(mask(leakyrelu(s_i+t_j))) @ h  per head, concat heads.

Decomposition (per head): exp(lrelu(e)) / exp(.2 s_i) = max(w_i v1_j, v2_j)
with w = exp(.8 s), v1 = exp(t), v2 = exp(.2 t); the 1/exp(.2 s_i) row scale
cancels in the softmax ratio.

Layout: adj is HOST-transposed per core (adjT[j, i]), so the binarized mask is
born in the [j-partition, i-free] orientation the attention tiles use — no PE
transposes, no mask copies.  Per j-tile:
  q_h   = tensor_scalar(wb_h, *v1_jh, max v2_jh)        (DVE, 4x mode)
  heads 0,1: pt = q * m      (one 2-head broadcast TT on DVE, 2x mode)
  heads 2,3: pt = (adjT > .5) * q   (fused scalar_tensor_tensor on GPSIMD,
             reading the RAW fp32 adj tile - no mask materialization)
  pair matmuls vs Vpack = 0.0625*[h|1] accumulate num/den in PSUM.
s/t scores come from host-folded WaS = W@a_src, WaD = W@a_dst ([D,H]) so the
[HF,N] feature intermediate is never built.  Finalize: transpose via 33x33
identity matmuls, reciprocal_approx_fast, one fused bias matmul, single
full-width output DMA.
"""
import os
import sys
import numpy as np

for _p in ("/opt/trn_rl_repo",):
    if _p not in sys.path:
        sys.path.insert(0, _p)

B, N, D, H, F = 4, 2048, 128, 4, 32
HF = H * F           # 128
IR = 1024            # i-rows per core
NJT = N // 128       # 16 j-tiles
NCORES = 8

# Load balance: jts where GPSIMD masks 2 heads (else just head 3)
POOL2 = {1, 4, 7, 10, 13}

_CACHE = {}


def build_nc(reps=1):
    import concourse.bacc as bacc
    import concourse.tile as tile
    from concourse import mybir

    f32, f16 = mybir.dt.float32, mybir.dt.float16
    Alu = mybir.AluOpType
    Act = mybir.ActivationFunctionType

    nc = bacc.Bacc(None, target_bir_lowering=False)

    xT_d   = nc.dram_tensor("xT",   [D, N],    f32, kind="ExternalInput")
    xiT_d  = nc.dram_tensor("xiT",  [D, IR],   f32, kind="ExternalInput")
    adjT_d = nc.dram_tensor("adjT", [N, IR],   f32, kind="ExternalInput")
    Wf_d   = nc.dram_tensor("Wfs",  [D, HF],   f32, kind="ExternalInput")
    WaS_d  = nc.dram_tensor("WaS",  [D, H],    f32, kind="ExternalInput")
    WaD_d  = nc.dram_tensor("WaD",  [D, H],    f32, kind="ExternalInput")
    bias_d = nc.dram_tensor("biasR", [1, HF],  f32, kind="ExternalInput")
    out_d  = nc.dram_tensor("out",  [IR, HF],  f32, kind="ExternalOutput")

    # host constants
    EY = np.zeros((4, 4 * 128), np.float16)
    for h in range(H):
        EY[h, h * 128:(h + 1) * 128] = 1.0
    EY_d = nc.inline_tensor(EY, "EYc")
    ID33_d = nc.inline_tensor(np.eye(33, dtype=np.float16), "id33c")

    adjT_r = adjT_d[:].rearrange("(s p) i -> p s i", p=128)

    with tile.TileContext(nc) as tc:
        cst_ctx = tc.tile_pool(name="cst", bufs=1)
        cst = cst_ctx.__enter__()
        try:
            xT    = cst.tile([D, N], f32)
            xiT   = cst.tile([D, IR], f32)
            Wf    = cst.tile([D, HF], f32)
            WaS   = cst.tile([D, H], f32)
            WaD   = cst.tile([D, H], f32)
            biasR = cst.tile([1, HF], f32)
            biasTE = cst.tile([64, 4, 33], f16)
            eyc   = cst.tile([4, 4 * 128], f16)
            id33  = cst.tile([33, 33], f16)
            sigB  = cst.tile([128, 1], f32)

            Wf16  = cst.tile([D, HF], f16)
            WaS16 = cst.tile([D, H], f16)
            WaD16 = cst.tile([D, H], f16)
            xT16  = cst.tile([D, N], f16)
            xiT16 = cst.tile([D, IR], f16)
            sZ4   = cst.tile([4, IR], f32)
            w16   = cst.tile([4, IR], f16)          # exp(.8 s)
            wb16  = cst.tile([128, H, IR], f16)     # per-head broadcast of w16
            tAll  = cst.tile([128, NJT, H], f32)
            tv1   = cst.tile([128, NJT, H], f32)    # exp(t)
            tv2   = cst.tile([128, NJT, H], f32)    # exp(.2 t)
            Vpack = cst.tile([128, NJT, H, 33], f16)
            out_sb = cst.tile([128, 8, HF], f32)

            nc.sync.dma_start(xiT[:], xiT_d[:])
            nc.sync.dma_start(xT[:], xT_d[:])
            nc.sync.dma_start(Wf[:], Wf_d[:])
            nc.sync.dma_start(WaS[:], WaS_d[:])
            nc.sync.dma_start(WaD[:], WaD_d[:])
            nc.sync.dma_start(biasR[:], bias_d[:])
            nc.sync.dma_start(eyc[:], EY_d[:])
            nc.sync.dma_start(id33[:], ID33_d[:])
            nc.vector.memset(sigB[:], -5e5)
            nc.vector.memset(biasTE[:], 0.0)
            nc.scalar.copy(
                biasTE[32:33, :, 0:32],
                biasR[:].rearrange("p (h f) -> p h f", h=H))

            # ---------------- prep ----------------
            nc.scalar.copy(Wf16[:], Wf[:])
            nc.scalar.copy(WaS16[:], WaS[:])
            nc.scalar.copy(WaD16[:], WaD[:])
            for k in range(2):
                nc.scalar.copy(xiT16[:, k * 512:(k + 1) * 512],
                               xiT[:, k * 512:(k + 1) * 512])
            for k in range(4):
                nc.scalar.copy(xT16[:, k * 512:(k + 1) * 512],
                               xT[:, k * 512:(k + 1) * 512])
            nc.gpsimd.memset(Vpack[:, :, :, 32:33], 0.0625)

            with tc.tile_pool(name="pp", bufs=3, space="PSUM") as pp:
                # s chain: xiT16 -> sZ4 -> w16 -> wb16
                for k in range(2):
                    ps = pp.tile([4, 512], f32, tag="pp")
                    nc.tensor.matmul(ps[:], WaS16[:], xiT16[:, k * 512:(k + 1) * 512],
                                     start=True, stop=True)
                    nc.scalar.copy(sZ4[:, k * 512:(k + 1) * 512], ps[:])
                nc.scalar.activation(w16[:], sZ4[:], Act.Exp, scale=0.8)
                for h in range(H):
                    for k in range(2):
                        ps = pp.tile([128, 512], f32, tag="pp")
                        nc.tensor.matmul(ps[:], eyc[:, h * 128:(h + 1) * 128],
                                         w16[:, k * 512:(k + 1) * 512],
                                         start=True, stop=True)
                        if h % 2 == 0:
                            nc.vector.tensor_copy(wb16[:, h, k * 512:(k + 1) * 512], ps[:])
                        else:
                            nc.scalar.copy(wb16[:, h, k * 512:(k + 1) * 512], ps[:])
                # t chain: per 4-jt group matmuls vs WaD16, then exps
                for g in range(4):
                    ps = pp.tile([128, 4 * H], f32, tag="pp")
                    for k4 in range(4):
                        jt = g * 4 + k4
                        nc.tensor.matmul(ps[:, k4 * H:(k4 + 1) * H],
                                         xT16[:, jt * 128:(jt + 1) * 128], WaD16[:],
                                         start=True, stop=True)
                    nc.scalar.copy(tAll[:, g * 4:(g + 1) * 4, :], ps[:])
                nc.scalar.activation(
                    tv1[:].rearrange("p a b -> p (a b)"),
                    tAll[:].rearrange("p a b -> p (a b)"), Act.Exp)
                nc.scalar.activation(
                    tv2[:].rearrange("p a b -> p (a b)"),
                    tAll[:].rearrange("p a b -> p (a b)"), Act.Exp, scale=0.2)
                # Vpack features: hJ = x@ (0.0625*W) in [j, hf] orientation
                for blk in range(8):
                    jt0 = 2 * blk
                    pv = pp.tile([128, 256], f32, tag="pp")
                    for d in range(2):
                        nc.tensor.matmul(
                            pv[:, d * 128:(d + 1) * 128],
                            xT16[:, (jt0 + d) * 128:(jt0 + d + 1) * 128],
                            Wf16[:], start=True, stop=True)
                    dst = Vpack[:, jt0:jt0 + 2, :, 0:32]
                    src = pv[:].rearrange("p (j h f) -> p j h f", j=2, h=H)
                    if blk % 2 == 0:
                        nc.vector.tensor_copy(dst, src)
                    else:
                        nc.scalar.copy(dst, src)

            # ---------------- main body (per rep) ----------------
            def emit_body():
                psg_ctx = tc.tile_pool(name="psg", bufs=1, space="PSUM")
                psg = psg_ctx.__enter__()
                pgA = psg.tile([97, IR], f32, tag="pgA", name="pgA")
                pgB = psg.tile([97, IR], f32, tag="pgB", name="pgB")
                pgt = {0: (pgA, 0), 1: (pgA, 64), 2: (pgB, 0), 3: (pgB, 64)}

                def pair_mm(h, jt, pt, hh):
                    tile_, off = pgt[h]
                    for k in range(2):
                        nc.tensor.matmul(
                            tile_[off:off + 33, k * 512:(k + 1) * 512],
                            Vpack[:, jt, h, :],
                            pt[:, hh, k * 512:(k + 1) * 512],
                            start=(jt == 0), stop=(jt == NJT - 1))

                with tc.tile_pool(name="adjp", bufs=3) as adjp, \
                     tc.tile_pool(name="mtp", bufs=2) as mtp, \
                     tc.tile_pool(name="qdp", bufs=2) as qdp, \
                     tc.tile_pool(name="qpp", bufs=2) as qpp, \
                     tc.tile_pool(name="ptdp", bufs=2) as ptdp, \
                     tc.tile_pool(name="ptpp", bufs=2) as ptpp:
                    for blk in range(8):
                        at = adjp.tile([128, 2, IR], f32, tag="adj")
                        nc.sync.dma_start(at[:], adjT_r[:, 2 * blk:2 * blk + 2, :])
                        mt = mtp.tile([128, 2, IR], f16, tag="mi")
                        nc.scalar.activation(
                            mt[:].rearrange("p a b -> p (a b)"),
                            at[:].rearrange("p a b -> p (a b)"),
                            Act.Sigmoid, bias=sigB[:, 0:1], scale=1e6)
                        for q in range(2):
                            jt = 2 * blk + q
                            nD = 2 if jt in POOL2 else 3
                            qD = qdp.tile([128, 3, IR], f16, tag="qD")
                            for h in range(nD):
                                nc.vector.tensor_scalar(
                                    qD[:, h, :], wb16[:, h, :],
                                    tv1[:, jt, h:h + 1], tv2[:, jt, h:h + 1],
                                    op0=Alu.mult, op1=Alu.max)
                            ptD = ptdp.tile([128, 3, IR], f16, tag="ptD")
                            nc.vector.tensor_tensor(
                                ptD[:, 0:nD, :], qD[:, 0:nD, :],
                                mt[:, q:q + 1, :].broadcast_to([128, nD, IR]),
                                op=Alu.mult)
                            qP = qpp.tile([128, 2, IR], f16, tag="qP")
                            for h in range(nD, 4):
                                nc.vector.tensor_scalar(
                                    qP[:, h - nD, :], wb16[:, h, :],
                                    tv1[:, jt, h:h + 1], tv2[:, jt, h:h + 1],
                                    op0=Alu.mult, op1=Alu.max)
                            ptP = ptpp.tile([128, 2, IR], f16, tag="ptP")
                            nP = 4 - nD
                            nc.gpsimd.tensor_tensor(
                                ptP[:, 0:nP, :], qP[:, 0:nP, :],
                                mt[:, q:q + 1, :].broadcast_to([128, nP, IR]),
                                op=Alu.mult)
                            for h in range(4):
                                if h < nD:
                                    pair_mm(h, jt, ptD, h)
                                else:
                                    pair_mm(h, jt, ptP, h - nD)

                # finalize
                ftp_ctx = tc.tile_pool(name="ftp", bufs=2, space="PSUM")
                ftp = ftp_ctx.__enter__()
                ndp_ctx = tc.tile_pool(name="ndp", bufs=4)
                ndp = ndp_ctx.__enter__()
                try:
                    for h in range(4):
                        tile_, off = pgt[h]
                        numD = ndp.tile([33, IR], f16, tag="numD")
                        nc.scalar.copy(numD[:], tile_[off:off + 33, :])
                        tpA = ftp.tile([128, 8, 33], f32, tag="tpA")
                        for c in range(8):
                            nc.tensor.matmul(tpA[:, c, :],
                                             numD[:, c * 128:(c + 1) * 128],
                                             id33[:], start=True, stop=False)
                            nc.tensor.matmul(tpA[:, c, :],
                                             numD[32:33, c * 128:(c + 1) * 128],
                                             biasTE[32:33, h, :],
                                             start=False, stop=True)
                        rdT = ndp.tile([128, 8, 1], f32, tag="rdT")
                        nc.vector.reciprocal_approx_fast(rdT[:], tpA[:, :, 32:33])
                        nc.vector.tensor_tensor(
                            out_sb[:, :, h * 32:(h + 1) * 32], tpA[:, :, 0:32],
                            rdT[:, :, 0:1].broadcast_to([128, 8, 32]), op=Alu.mult)
                    nc.sync.dma_start(
                        out_d[:].rearrange("(s p) f -> p s f", p=128), out_sb[:])
                finally:
                    ndp_ctx.__exit__(None, None, None)
                    ftp_ctx.__exit__(None, None, None)
                    psg_ctx.__exit__(None, None, None)

            for _rep in range(reps):
                emit_body()
        finally:
            cst_ctx.__exit__(None, None, None)

    nc.compile()
    return nc


def _prepare_in_maps(x, adj, W, a_src, a_dst, bias):
    x = np.ascontiguousarray(np.asarray(x, dtype=np.float32))
    adj = np.asarray(adj, dtype=np.float32)
    W = np.asarray(W, dtype=np.float32)
    a_src = np.asarray(a_src, dtype=np.float32)
    a_dst = np.asarray(a_dst, dtype=np.float32)
    bias = np.asarray(bias, dtype=np.float32)

    Wf = np.ascontiguousarray(W.reshape(D, HF)) * 0.0625
    WaS = np.ascontiguousarray(np.einsum("dhf,hf->dh", W.reshape(D, H, F), a_src))
    WaD = np.ascontiguousarray(np.einsum("dhf,hf->dh", W.reshape(D, H, F), a_dst))
    biasRh = np.ascontiguousarray(bias.reshape(1, HF))

    in_maps = []
    for c in range(NCORES):
        b, cc = c // 2, c % 2
        i0 = cc * IR
        in_maps.append({
            "xT": np.ascontiguousarray(x[b].T),
            "xiT": np.ascontiguousarray(x[b, i0:i0 + IR].T),
            "adjT": np.ascontiguousarray(adj[b, i0:i0 + IR, :].T),
            "Wfs": Wf,
            "WaS": WaS,
            "WaD": WaD,
            "biasR": biasRh,
        })
    return in_maps


def run(inputs, trace=False, trace_cores=None):
    from concourse.bass_utils import run_bass_kernel_spmd
    if "nc" not in _CACHE:
        _CACHE["nc"] = build_nc()
    nc = _CACHE["nc"]
    in_maps = _prepare_in_maps(**inputs)
    kw = {}
    if trace:
        kw = dict(trace=True, trace_cores=trace_cores or [0])
    res = run_bass_kernel_spmd(nc, in_maps, list(range(NCORES)), **kw)
    out = np.zeros((B, N, HF), np.float32)
    for c in range(NCORES):
        b, cc = c // 2, c % 2
        out[b, cc * IR:(cc + 1) * IR, :] = res.results[c]["out"]
    return out, res


def kernel(**inputs):
    out, _ = run(inputs, trace=False)
    return out


# revision 58
# speedup vs baseline: 1.2969x; 1.2969x over previous
"""Batched GAT kernel for 8 Trainium2 NeuronCores.

Math: out[b,i,:] = softmax_j(mask(leakyrelu(s_i+t_j))) @ h  per head, concat heads.

Decomposition (per head): exp(lrelu(e)) / exp(.2 s_i) = max(w_i v1_j, v2_j)
with w = exp(.8 s), v1 = exp(t), v2 = exp(.2 t); the 1/exp(.2 s_i) row scale
cancels in the softmax ratio.

Layout: adj is HOST-transposed per core (adjT[j, i]), so the binarized mask is
born in the [j-partition, i-free] orientation the attention tiles use — no PE
transposes, no mask copies.  Per j-tile:
  q_h = tensor_scalar(wb_h, *v1_jh, max v2_jh)   (DVE, 4x mode)
  pt  = q * m  — split between one multi-head broadcast TT on DVE (2x mode)
        and a GPSIMD TT for 0-2 heads per tile (POOLH load-balance knob)
  pair matmuls vs Vpack = 0.0625*[h|1] accumulate num/den in PSUM, emitted in
  16-matmul per-block clusters to keep the PE p-state ramped.
s/t scores come from host-folded WaS = W@a_src, WaD = W@a_dst ([D,H]); the x
feature stream is processed in 512-column chunks interleaved with the adj
stream so first-tile compute starts ~6us in.  Finalize: transpose via 33x33
identity matmuls, reciprocal_approx_fast, fused bias matmul, single 512B-row
output DMA.
"""
import os
import sys
import numpy as np

for _p in ("/opt/trn_rl_repo",):
    if _p not in sys.path:
        sys.path.insert(0, _p)

B, N, D, H, F = 4, 2048, 128, 4, 32
HF = H * F           # 128
IR = 1024            # i-rows per core
NJT = N // 128       # 16 j-tiles
NCORES = 8

import json as _json

def _knob(name, default):
    v = os.environ.get(name)
    return _json.loads(v) if v else default

# adj blocks are processed in this order; the accumulation group opens on the
# first entry's j-tiles and closes on the last (process order is free since
# the PSUM accumulate is a sum)
BLK_ORDER = _knob("K_BORD", [7, 0, 1, 2, 3, 4, 5, 6])
# pool-head count per position in BLK_ORDER (load balance DVE vs GPSIMD;
# last positions 0 so GPSIMD drains before the finale)
POOLH_SEQ = _knob("K_PSEQ", [1, 1, 1, 1, 2, 1, 2, 1, 2, 1, 2, 1, 1, 1, 1, 0])
# blocks (by position) whose mask is binarized on GPSIMD (idle during
# startup) instead of waiting for the Act sigmoid table; DVE_BIN likewise on
# the vector engine (fills its pre-steady-state stall)
POOL_BIN = set(_knob("K_PBIN", [0, 1]))
DVE_BIN = set(_knob("K_DBIN", []))
BUFS = _knob("K_BUFS", {})
BUFS = {"adjp": 3, "mtp": 4, "qdp": 3, "qpp": 4, "ptdp": 4, "ptpp": 4, **BUFS}

_CACHE = {}


def build_nc(reps=1):
    import concourse.bacc as bacc
    import concourse.tile as tile
    from concourse import mybir

    f32, f16 = mybir.dt.float32, mybir.dt.float16
    Alu = mybir.AluOpType
    Act = mybir.ActivationFunctionType

    nc = bacc.Bacc(None, target_bir_lowering=False)

    xT_d   = nc.dram_tensor("xT",   [D, N],    f32, kind="ExternalInput")
    xiT_d  = nc.dram_tensor("xiT",  [D, IR],   f32, kind="ExternalInput")
    adjT_d = nc.dram_tensor("adjT", [N, IR],   f32, kind="ExternalInput")
    Wf_d   = nc.dram_tensor("Wfs",  [D, HF],   f32, kind="ExternalInput")
    WaS_d  = nc.dram_tensor("WaS",  [D, H],    f32, kind="ExternalInput")
    WaD_d  = nc.dram_tensor("WaD",  [D, H],    f32, kind="ExternalInput")
    bias_d = nc.dram_tensor("biasR", [1, HF],  f32, kind="ExternalInput")
    out_d  = nc.dram_tensor("out",  [IR, HF],  f32, kind="ExternalOutput")

    EY = np.zeros((4, 4 * 128), np.float16)
    for h in range(H):
        EY[h, h * 128:(h + 1) * 128] = 1.0
    EY_d = nc.inline_tensor(EY, "EYc")
    ID33_d = nc.inline_tensor(np.eye(33, dtype=np.float16), "id33c")

    adjT_r = adjT_d[:].rearrange("(s p) i -> p s i", p=128)

    with tile.TileContext(nc) as tc:
        cst_ctx = tc.tile_pool(name="cst", bufs=1)
        cst = cst_ctx.__enter__()
        try:
            xiT   = cst.tile([D, IR], f32)
            xT    = cst.tile([D, N], f32)
            Wf    = cst.tile([D, HF], f32)
            WaS   = cst.tile([D, H], f32)
            WaD   = cst.tile([D, H], f32)
            biasR = cst.tile([1, HF], f32)
            biasTE = cst.tile([64, 4, 33], f16)
            eyc   = cst.tile([4, 4 * 128], f16)
            id33  = cst.tile([33, 33], f16)
            sigB  = cst.tile([128, 1], f32)

            Wf16  = cst.tile([D, HF], f16)
            WaS16 = cst.tile([D, H], f16)
            WaD16 = cst.tile([D, H], f16)
            xT16  = cst.tile([D, N], f16)
            xiT16 = cst.tile([D, IR], f16)
            sZ4   = cst.tile([4, IR], f32)
            w16   = cst.tile([4, IR], f16)          # exp(.8 s)
            wb16  = cst.tile([128, H, IR], f16)     # per-head broadcast of w16
            tAll  = cst.tile([128, NJT, H], f32)
            tv1   = cst.tile([128, NJT, H], f32)    # exp(t)
            tv2   = cst.tile([128, NJT, H], f32)    # exp(.2 t)
            Vpack = cst.tile([128, NJT, H, 33], f16)
            out_sb = cst.tile([128, 8, HF], f32)

            # -------- DMA order: score path first, adj stream next --------
            nc.sync.dma_start(xiT[:], xiT_d[:])
            nc.sync.dma_start(WaS[:], WaS_d[:])
            # (adjT chunk DMAs are issued inside the block loop; pool bufs=3
            # lets the first three queue right behind these)
            nc.sync.dma_start(Wf[:], Wf_d[:])
            nc.sync.dma_start(WaD[:], WaD_d[:])
            nc.sync.dma_start(eyc[:], EY_d[:])
            nc.vector.memset(biasTE[:], 0.0)
            nc.gpsimd.memset(Vpack[:, :, :, 32:33], 0.0625)

            # -------- score/weight prep (DVE does the copies: it is idle
            # during startup and the bottleneck later) --------
            nc.vector.tensor_copy(WaS16[:], WaS[:])
            nc.vector.tensor_copy(WaD16[:], WaD[:])
            nc.vector.tensor_copy(Wf16[:], Wf[:])
            for k in range(2):
                nc.vector.tensor_copy(xiT16[:, k * 512:(k + 1) * 512],
                                      xiT[:, k * 512:(k + 1) * 512])

            pp_ctx = tc.tile_pool(name="pp", bufs=2, space="PSUM")
            pp = pp_ctx.__enter__()
            for k in range(2):
                ps = pp.tile([4, 512], f32, tag="pp")
                nc.tensor.matmul(ps[:], WaS16[:], xiT16[:, k * 512:(k + 1) * 512],
                                 start=True, stop=True)
                nc.vector.tensor_copy(sZ4[:, k * 512:(k + 1) * 512], ps[:])
            nc.scalar.activation(w16[:], sZ4[:], Act.Exp, scale=0.8)
            # w broadcasts immediately (DVE is idle now; h3 first since the
            # pool-side TS ops consume it soonest)
            for h in (3, 2, 1, 0):
                for k in range(2):
                    ps = pp.tile([128, 512], f32, tag="pp")
                    nc.tensor.matmul(ps[:], eyc[:, h * 128:(h + 1) * 128],
                                     w16[:, k * 512:(k + 1) * 512],
                                     start=True, stop=True)
                    nc.vector.tensor_copy(wb16[:, h, k * 512:(k + 1) * 512],
                                          ps[:])

            def xchunk_scores(g):
                # x columns [512g, 512(g+1)): t-scores + exps (gates sigmoids)
                nc.sync.dma_start(xT[:, g * 512:(g + 1) * 512],
                                  xT_d[:, g * 512:(g + 1) * 512])
                nc.scalar.copy(xT16[:, g * 512:(g + 1) * 512],
                               xT[:, g * 512:(g + 1) * 512])
                ps = pp.tile([128, 4 * H], f32, tag="pp")
                for k4 in range(4):
                    jt = g * 4 + k4
                    nc.tensor.matmul(ps[:, k4 * H:(k4 + 1) * H],
                                     xT16[:, jt * 128:(jt + 1) * 128], WaD16[:],
                                     start=True, stop=True)
                nc.scalar.copy(tAll[:, g * 4:(g + 1) * 4, :], ps[:])
                nc.scalar.activation(
                    tv1[:, g * 4:(g + 1) * 4, :].rearrange("p a b -> p (a b)"),
                    tAll[:, g * 4:(g + 1) * 4, :].rearrange("p a b -> p (a b)"),
                    Act.Exp)
                nc.scalar.activation(
                    tv2[:, g * 4:(g + 1) * 4, :].rearrange("p a b -> p (a b)"),
                    tAll[:, g * 4:(g + 1) * 4, :].rearrange("p a b -> p (a b)"),
                    Act.Exp, scale=0.2)

            def xchunk_vpack(blk):
                jt0 = 2 * blk
                pv = pp.tile([128, 256], f32, tag="pp")
                for d in range(2):
                    nc.tensor.matmul(
                        pv[:, d * 128:(d + 1) * 128],
                        xT16[:, (jt0 + d) * 128:(jt0 + d + 1) * 128],
                        Wf16[:], start=True, stop=True)
                nc.scalar.copy(
                    Vpack[:, jt0:jt0 + 2, :, 0:32],
                    pv[:].rearrange("p (j h f) -> p j h f", j=2, h=H))

            # late, fin-only params
            ID33_dma = lambda: (nc.sync.dma_start(id33[:], ID33_d[:]),
                                nc.sync.dma_start(biasR[:], bias_d[:]),
                                nc.scalar.copy(
                                    biasTE[32:33, :, 0:32],
                                    biasR[:].rearrange("p (h f) -> p h f", h=H)))

            def gate_sigmoid():
                # sigB is the sigmoid bias; producing it from the last exp
                # output (a Copy, so no table change) forces every Exp before
                # the first Sigmoid -> exactly one exp->sigmoid table switch.
                for g in range(4):
                    xchunk_scores(g)
                nc.scalar.activation(sigB[:], tv2[:, 15, 3:4], Act.Copy,
                                     bias=-5e5, scale=0.0)
                for blk in BLK_ORDER:
                    xchunk_vpack(blk)

            # ---------------- main body (per rep) ----------------
            def emit_body():
                psg_ctx = tc.tile_pool(name="psg", bufs=1, space="PSUM")
                psg = psg_ctx.__enter__()
                pgA = psg.tile([97, IR], f32, tag="pgA", name="pgA")
                pgB = psg.tile([97, IR], f32, tag="pgB", name="pgB")
                pgt = {0: (pgA, 0), 1: (pgA, 64), 2: (pgB, 0), 3: (pgB, 64)}

                def pair_mm(h, jt, pt, hh, pos):
                    tile_, off = pgt[h]
                    for k in range(2):
                        nc.tensor.matmul(
                            tile_[off:off + 33, k * 512:(k + 1) * 512],
                            Vpack[:, jt, h, :],
                            pt[:, hh, k * 512:(k + 1) * 512],
                            start=(pos == 0), stop=(pos == NJT - 1))

                with tc.tile_pool(name="adjp", bufs=BUFS["adjp"]) as adjp, \
                     tc.tile_pool(name="mtp", bufs=BUFS["mtp"]) as mtp, \
                     tc.tile_pool(name="qdp", bufs=BUFS["qdp"]) as qdp, \
                     tc.tile_pool(name="qpp", bufs=BUFS["qpp"]) as qpp, \
                     tc.tile_pool(name="ptdp", bufs=BUFS["ptdp"]) as ptdp, \
                     tc.tile_pool(name="ptpp", bufs=BUFS["ptpp"]) as ptpp:
                    # first adj chunks + their masks run pre-gate:
                    # DMA early, binarize on the (idle) GPSIMD
                    early = {}
                    for p_ in sorted(POOL_BIN):
                        blk = BLK_ORDER[p_]
                        at = adjp.tile([128, 2, IR], f32, tag="adj")
                        nc.sync.dma_start(at[:], adjT_r[:, 2 * blk:2 * blk + 2, :])
                        mt = mtp.tile([128, 2, IR], f16, tag="mi")
                        nc.gpsimd.tensor_scalar(
                            mt[:], at[:], 0.5, None, op0=Alu.is_gt)
                        early[blk] = (at, mt)
                    gate_sigmoid()
                    pend = []
                    for bpos, blk in enumerate(BLK_ORDER):
                        if blk in early:
                            at, mt = early[blk]
                        else:
                            at = adjp.tile([128, 2, IR], f32, tag="adj")
                            nc.sync.dma_start(at[:],
                                              adjT_r[:, 2 * blk:2 * blk + 2, :])
                            mt = mtp.tile([128, 2, IR], f16, tag="mi")
                            if bpos in DVE_BIN:
                                nc.vector.tensor_scalar(
                                    mt[:], at[:], 0.5, None, op0=Alu.is_gt)
                            else:
                                nc.scalar.activation(
                                    mt[:].rearrange("p a b -> p (a b)"),
                                    at[:].rearrange("p a b -> p (a b)"),
                                    Act.Sigmoid, bias=sigB[:, 0:1], scale=1e6)
                        for q in range(2):
                            jt = 2 * blk + q
                            pos = 2 * bpos + q
                            nP = POOLH_SEQ[pos]
                            nD = 4 - nP
                            # pool-side first so GPSIMD starts early
                            if nP:
                                qP = qpp.tile([128, 2, IR], f16, tag="qP")
                                for h in range(nD, 4):
                                    nc.vector.tensor_scalar(
                                        qP[:, h - nD, :], wb16[:, h, :],
                                        tv1[:, jt, h:h + 1], tv2[:, jt, h:h + 1],
                                        op0=Alu.mult, op1=Alu.max)
                                ptP = ptpp.tile([128, 2, IR], f16, tag="ptP")
                                nc.gpsimd.tensor_tensor(
                                    ptP[:, 0:nP, :], qP[:, 0:nP, :],
                                    mt[:, q:q + 1, :].broadcast_to([128, nP, IR]),
                                    op=Alu.mult)
                            else:
                                ptP = None
                            qD = qdp.tile([128, 4, IR], f16, tag="qD")
                            for h in range(nD):
                                nc.vector.tensor_scalar(
                                    qD[:, h, :], wb16[:, h, :],
                                    tv1[:, jt, h:h + 1], tv2[:, jt, h:h + 1],
                                    op0=Alu.mult, op1=Alu.max)
                            ptD = ptdp.tile([128, 4, IR], f16, tag="ptD")
                            nc.vector.tensor_tensor(
                                ptD[:, 0:nD, :], qD[:, 0:nD, :],
                                mt[:, q:q + 1, :].broadcast_to([128, nD, IR]),
                                op=Alu.mult)
                            pend.append((jt, pos, nD, ptD, ptP))
                        if bpos == 6:
                            ID33_dma()
                        # 32-matmul PE clusters (every 2 blocks) keep the PE
                        # p-state ramped; the last block goes alone, DVE-fed
                        # matmuls first and h0/h1 before h2/h3, so the
                        # finalization overlaps the trailing mask work
                        if bpos % 2 == 1 or bpos >= 6:
                            if bpos == 7:
                                order = sorted(
                                    ((h, e) for h in range(4) for e in pend),
                                    key=lambda he: (he[0] >= 4 - he[1][2], he[0]))
                            else:
                                order = [(h, e) for e in pend for h in range(4)]
                            for h, (jt, pos, nD, ptD, ptP) in order:
                                if h < nD:
                                    pair_mm(h, jt, ptD, h, pos)
                                else:
                                    pair_mm(h, jt, ptP, h - nD, pos)
                            pend = []

                # finalize
                ftp_ctx = tc.tile_pool(name="ftp", bufs=2, space="PSUM")
                ftp = ftp_ctx.__enter__()
                ndp_ctx = tc.tile_pool(name="ndp", bufs=4)
                ndp = ndp_ctx.__enter__()
                try:
                    for h in range(4):
                        tile_, off = pgt[h]
                        numD = ndp.tile([33, IR], f16, tag="numD")
                        nc.scalar.copy(numD[:], tile_[off:off + 33, :])
                        tpA = ftp.tile([128, 8, 33], f32, tag="tpA")
                        for c in range(8):
                            nc.tensor.matmul(tpA[:, c, :],
                                             numD[:, c * 128:(c + 1) * 128],
                                             id33[:], start=True, stop=False)
                            nc.tensor.matmul(tpA[:, c, :],
                                             numD[32:33, c * 128:(c + 1) * 128],
                                             biasTE[32:33, h, :],
                                             start=False, stop=True)
                        rdT = ndp.tile([128, 8, 1], f32, tag="rdT")
                        nc.vector.reciprocal_approx_fast(rdT[:], tpA[:, :, 32:33])
                        nc.vector.tensor_tensor(
                            out_sb[:, :, h * 32:(h + 1) * 32], tpA[:, :, 0:32],
                            rdT[:, :, 0:1].broadcast_to([128, 8, 32]), op=Alu.mult)
                    nc.sync.dma_start(
                        out_d[:].rearrange("(s p) f -> p s f", p=128), out_sb[:])
                finally:
                    ndp_ctx.__exit__(None, None, None)
                    ftp_ctx.__exit__(None, None, None)
                    psg_ctx.__exit__(None, None, None)

            for _rep in range(reps):
                emit_body()
            pp_ctx.__exit__(None, None, None)
        finally:
            cst_ctx.__exit__(None, None, None)

    nc.compile()
    return nc


def _prepare_in_maps(x, adj, W, a_src, a_dst, bias):
    x = np.ascontiguousarray(np.asarray(x, dtype=np.float32))
    adj = np.asarray(adj, dtype=np.float32)
    W = np.asarray(W, dtype=np.float32)
    a_src = np.asarray(a_src, dtype=np.float32)
    a_dst = np.asarray(a_dst, dtype=np.float32)
    bias = np.asarray(bias, dtype=np.float32)

    Wf = np.ascontiguousarray(W.reshape(D, HF)) * 0.0625
    WaS = np.ascontiguousarray(np.einsum("dhf,hf->dh", W.reshape(D, H, F), a_src))
    WaD = np.ascontiguousarray(np.einsum("dhf,hf->dh", W.reshape(D, H, F), a_dst))
    biasRh = np.ascontiguousarray(bias.reshape(1, HF))

    in_maps = []
    for c in range(NCORES):
        b, cc = c // 2, c % 2
        i0 = cc * IR
        in_maps.append({
            "xT": np.ascontiguousarray(x[b].T),
            "xiT": np.ascontiguousarray(x[b, i0:i0 + IR].T),
            "adjT": np.ascontiguousarray(adj[b, i0:i0 + IR, :].T),
            "Wfs": Wf,
            "WaS": WaS,
            "WaD": WaD,
            "biasR": biasRh,
        })
    return in_maps


def run(inputs, trace=False, trace_cores=None):
    from concourse.bass_utils import run_bass_kernel_spmd
    if "nc" not in _CACHE:
        _CACHE["nc"] = build_nc()
    nc = _CACHE["nc"]
    in_maps = _prepare_in_maps(**inputs)
    kw = {}
    if trace:
        kw = dict(trace=True, trace_cores=trace_cores or [0])
    res = run_bass_kernel_spmd(nc, in_maps, list(range(NCORES)), **kw)
    out = np.zeros((B, N, HF), np.float32)
    for c in range(NCORES):
        b, cc = c // 2, c % 2
        out[b, cc * IR:(cc + 1) * IR, :] = res.results[c]["out"]
    return out, res


def kernel(**inputs):
    out, _ = run(inputs, trace=False)
    return out


# revision 66
# speedup vs baseline: 1.3361x; 1.0302x over previous
"""Batched GAT kernel for 8 Trainium2 NeuronCores.

Math: out[b,i,:] = softmax_j(mask(leakyrelu(s_i+t_j))) @ h  per head, concat heads.

Decomposition (per head): exp(lrelu(e)) / exp(.2 s_i) = max(w_i v1_j, v2_j)
with w = exp(.8 s), v1 = exp(t), v2 = exp(.2 t); the 1/exp(.2 s_i) row scale
cancels in the softmax ratio.

Layout: adj is HOST-transposed per core (adjT[j, i]), so the binarized mask is
born in the [j-partition, i-free] orientation the attention tiles use — no PE
transposes, no mask copies.  Per j-tile:
  q_h = tensor_scalar(wb_h, *v1_jh, max v2_jh)   (DVE, 4x mode)
  pt  = q * m  — split between one multi-head broadcast TT on DVE (2x mode)
        and a GPSIMD TT for 0-2 heads per tile (POOLH load-balance knob)
  pair matmuls vs Vpack = 0.0625*[h|1] accumulate num/den in PSUM, emitted in
  16-matmul per-block clusters to keep the PE p-state ramped.
s/t scores come from host-folded WaS = W@a_src, WaD = W@a_dst ([D,H]); the x
feature stream is processed in 512-column chunks interleaved with the adj
stream so first-tile compute starts ~6us in.  Finalize: transpose via 33x33
identity matmuls, reciprocal_approx_fast, fused bias matmul, single 512B-row
output DMA.
"""
import os
import sys
import numpy as np

for _p in ("/opt/trn_rl_repo",):
    if _p not in sys.path:
        sys.path.insert(0, _p)

B, N, D, H, F = 4, 2048, 128, 4, 32
HF = H * F           # 128
IR = 1024            # i-rows per core
NJT = N // 128       # 16 j-tiles
NCORES = 8

import json as _json

def _knob(name, default):
    v = os.environ.get(name)
    return _json.loads(v) if v else default

# adj blocks are processed in this order; the accumulation group opens on the
# first entry's j-tiles and closes on the last (process order is free since
# the PSUM accumulate is a sum)
BLK_ORDER = _knob("K_BORD", [7, 0, 1, 2, 3, 4, 5, 6])
# pool-head count per position in BLK_ORDER (load balance DVE vs GPSIMD;
# last positions 0 so GPSIMD drains before the finale)
POOLH_SEQ = _knob("K_PSEQ", [1, 1, 1, 1, 2, 1, 2, 1, 2, 1, 1, 2, 1, 1, 1, 1])
# blocks (by position) whose mask is binarized on GPSIMD (idle during
# startup) instead of waiting for the Act sigmoid table; DVE_BIN likewise on
# the vector engine (fills its pre-steady-state stall)
POOL_BIN = set(_knob("K_PBIN", [0, 1]))
DVE_BIN = set(_knob("K_DBIN", []))
# positions where heads 0/1 use the Act-engine relu route:
# p~ = v2 + relu(v1*w - v2); the relu part goes through the normal masked
# pair matmuls and the v2 part is recovered by a mask-matmul against the
# v2-scaled feature pack (same PSUM accumulation group)
RELU_POS = set(_knob("K_RELU", [8, 9, 10, 11, 12, 13]))
BUFS = _knob("K_BUFS", {})
BUFS = {"adjp": 3, "mtp": 4, "qdp": 3, "qpp": 4, "ptdp": 4, "ptpp": 4, **BUFS}

_CACHE = {}


def build_nc(reps=1):
    import concourse.bacc as bacc
    import concourse.tile as tile
    from concourse import mybir

    f32, f16 = mybir.dt.float32, mybir.dt.float16
    Alu = mybir.AluOpType
    Act = mybir.ActivationFunctionType

    nc = bacc.Bacc(None, target_bir_lowering=False)

    xT_d   = nc.dram_tensor("xT",   [D, N],    f32, kind="ExternalInput")
    xiT_d  = nc.dram_tensor("xiT",  [D, IR],   f32, kind="ExternalInput")
    adjT_d = nc.dram_tensor("adjT", [N, IR],   f32, kind="ExternalInput")
    Wf_d   = nc.dram_tensor("Wfs",  [D, HF],   f32, kind="ExternalInput")
    WaS_d  = nc.dram_tensor("WaS",  [D, H],    f32, kind="ExternalInput")
    WaD_d  = nc.dram_tensor("WaD",  [D, H],    f32, kind="ExternalInput")
    bias_d = nc.dram_tensor("biasR", [1, HF],  f32, kind="ExternalInput")
    out_d  = nc.dram_tensor("out",  [IR, HF],  f32, kind="ExternalOutput")

    EY = np.zeros((4, 4 * 128), np.float16)
    for h in range(H):
        EY[h, h * 128:(h + 1) * 128] = 1.0
    EY_d = nc.inline_tensor(EY, "EYc")
    ID33_d = nc.inline_tensor(np.eye(33, dtype=np.float16), "id33c")

    adjT_r = adjT_d[:].rearrange("(s p) i -> p s i", p=128)

    with tile.TileContext(nc) as tc:
        cst_ctx = tc.tile_pool(name="cst", bufs=1)
        cst = cst_ctx.__enter__()
        try:
            xiT   = cst.tile([D, IR], f32)
            xT    = cst.tile([D, N], f32)
            Wf    = cst.tile([D, HF], f32)
            WaS   = cst.tile([D, H], f32)
            WaD   = cst.tile([D, H], f32)
            biasR = cst.tile([1, HF], f32)
            biasTE = cst.tile([64, 4, 33], f16)
            eyc   = cst.tile([4, 4 * 128], f16)
            id33  = cst.tile([33, 33], f16)
            sigB  = cst.tile([128, 1], f32)

            Wf16  = cst.tile([D, HF], f16)
            WaS16 = cst.tile([D, H], f16)
            WaD16 = cst.tile([D, H], f16)
            xT16  = cst.tile([D, N], f16)
            xiT16 = cst.tile([D, IR], f16)
            sZ4   = cst.tile([4, IR], f32)
            w16   = cst.tile([4, IR], f16)          # exp(.8 s)
            wb16  = cst.tile([128, H, IR], f16)     # per-head broadcast of w16
            tAll  = cst.tile([128, NJT, H], f32)
            tv1   = cst.tile([128, NJT, H], f32)    # exp(t)
            tv2   = cst.tile([128, NJT, H], f32)    # exp(.2 t)
            tv2n  = cst.tile([128, NJT, H], f32)    # -exp(.2 t)
            Vpack = cst.tile([128, NJT, H, 33], f16)
            g2pP  = cst.tile([128, NJT, 97], f16)   # v2-scaled packs, pg rows
            out_sb = cst.tile([128, 8, HF], f32)

            # -------- DMA order: score path first, adj stream next --------
            nc.sync.dma_start(xiT[:], xiT_d[:])
            nc.sync.dma_start(WaS[:], WaS_d[:])
            # (adjT chunk DMAs are issued inside the block loop; pool bufs=3
            # lets the first three queue right behind these)
            nc.sync.dma_start(Wf[:], Wf_d[:])
            nc.sync.dma_start(WaD[:], WaD_d[:])
            nc.sync.dma_start(eyc[:], EY_d[:])
            nc.vector.memset(biasTE[:], 0.0)
            nc.gpsimd.memset(Vpack[:, :, :, 32:33], 0.0625)

            # -------- score/weight prep (DVE does the copies: it is idle
            # during startup and the bottleneck later) --------
            nc.vector.tensor_copy(WaS16[:], WaS[:])
            nc.vector.tensor_copy(WaD16[:], WaD[:])
            nc.vector.tensor_copy(Wf16[:], Wf[:])
            for k in range(2):
                nc.vector.tensor_copy(xiT16[:, k * 512:(k + 1) * 512],
                                      xiT[:, k * 512:(k + 1) * 512])

            pp_ctx = tc.tile_pool(name="pp", bufs=2, space="PSUM")
            pp = pp_ctx.__enter__()
            for k in range(2):
                ps = pp.tile([4, 512], f32, tag="pp")
                nc.tensor.matmul(ps[:], WaS16[:], xiT16[:, k * 512:(k + 1) * 512],
                                 start=True, stop=True)
                nc.vector.tensor_copy(sZ4[:, k * 512:(k + 1) * 512], ps[:])
            nc.scalar.activation(w16[:], sZ4[:], Act.Exp, scale=0.8)
            # w broadcasts immediately (DVE is idle now; h3 first since the
            # pool-side TS ops consume it soonest)
            for h in (3, 2, 1, 0):
                for k in range(2):
                    ps = pp.tile([128, 512], f32, tag="pp")
                    nc.tensor.matmul(ps[:], eyc[:, h * 128:(h + 1) * 128],
                                     w16[:, k * 512:(k + 1) * 512],
                                     start=True, stop=True)
                    nc.vector.tensor_copy(wb16[:, h, k * 512:(k + 1) * 512],
                                          ps[:])

            def xchunk_scores(g):
                # x columns [512g, 512(g+1)): t-scores + exps (gates sigmoids)
                nc.sync.dma_start(xT[:, g * 512:(g + 1) * 512],
                                  xT_d[:, g * 512:(g + 1) * 512])
                nc.scalar.copy(xT16[:, g * 512:(g + 1) * 512],
                               xT[:, g * 512:(g + 1) * 512])
                ps = pp.tile([128, 4 * H], f32, tag="pp")
                for k4 in range(4):
                    jt = g * 4 + k4
                    nc.tensor.matmul(ps[:, k4 * H:(k4 + 1) * H],
                                     xT16[:, jt * 128:(jt + 1) * 128], WaD16[:],
                                     start=True, stop=True)
                nc.scalar.copy(tAll[:, g * 4:(g + 1) * 4, :], ps[:])
                nc.scalar.activation(
                    tv1[:, g * 4:(g + 1) * 4, :].rearrange("p a b -> p (a b)"),
                    tAll[:, g * 4:(g + 1) * 4, :].rearrange("p a b -> p (a b)"),
                    Act.Exp)
                nc.scalar.activation(
                    tv2[:, g * 4:(g + 1) * 4, :].rearrange("p a b -> p (a b)"),
                    tAll[:, g * 4:(g + 1) * 4, :].rearrange("p a b -> p (a b)"),
                    Act.Exp, scale=0.2)

            def xchunk_vpack(blk):
                jt0 = 2 * blk
                pv = pp.tile([128, 256], f32, tag="pp")
                for d in range(2):
                    nc.tensor.matmul(
                        pv[:, d * 128:(d + 1) * 128],
                        xT16[:, (jt0 + d) * 128:(jt0 + d + 1) * 128],
                        Wf16[:], start=True, stop=True)
                nc.scalar.copy(
                    Vpack[:, jt0:jt0 + 2, :, 0:32],
                    pv[:].rearrange("p (j h f) -> p j h f", j=2, h=H))

            # late, fin-only params
            ID33_dma = lambda: (nc.sync.dma_start(id33[:], ID33_d[:]),
                                nc.sync.dma_start(biasR[:], bias_d[:]),
                                nc.scalar.copy(
                                    biasTE[32:33, :, 0:32],
                                    biasR[:].rearrange("p (h f) -> p h f", h=H)))

            def gate_sigmoid():
                # sigB is the sigmoid bias; producing it from the last exp
                # output (a Copy, so no table change) forces every Exp before
                # the first Sigmoid -> exactly one exp->sigmoid table switch.
                for g in range(4):
                    xchunk_scores(g)
                nc.scalar.activation(sigB[:], tv2[:, 15, 3:4], Act.Copy,
                                     bias=-5e5, scale=0.0)
                nc.vector.tensor_scalar(
                    tv2n[:].rearrange("p a b -> p (a b)"),
                    tv2[:].rearrange("p a b -> p (a b)"), -1.0, None,
                    op0=Alu.mult)
                if RELU_POS:
                    nc.vector.memset(g2pP[:, :, 33:64], 0.0)
                for blk in BLK_ORDER:
                    xchunk_vpack(blk)

            # ---------------- main body (per rep) ----------------
            def emit_body():
                psg_ctx = tc.tile_pool(name="psg", bufs=1, space="PSUM")
                psg = psg_ctx.__enter__()
                pgA = psg.tile([97, IR], f32, tag="pgA", name="pgA")
                pgB = psg.tile([97, IR], f32, tag="pgB", name="pgB")
                pgt = {0: (pgA, 0), 1: (pgA, 64), 2: (pgB, 0), 3: (pgB, 64)}

                def pair_mm(h, jt, pt, hh, pos):
                    tile_, off = pgt[h]
                    for k in range(2):
                        nc.tensor.matmul(
                            tile_[off:off + 33, k * 512:(k + 1) * 512],
                            Vpack[:, jt, h, :],
                            pt[:, hh, k * 512:(k + 1) * 512],
                            start=(pos == 0), stop=(pos == NJT - 1))

                with tc.tile_pool(name="adjp", bufs=BUFS["adjp"]) as adjp, \
                     tc.tile_pool(name="mtp", bufs=BUFS["mtp"]) as mtp, \
                     tc.tile_pool(name="qdp", bufs=BUFS["qdp"]) as qdp, \
                     tc.tile_pool(name="qpp", bufs=BUFS["qpp"]) as qpp, \
                     tc.tile_pool(name="ptdp", bufs=BUFS["ptdp"]) as ptdp, \
                     tc.tile_pool(name="ptpp", bufs=BUFS["ptpp"]) as ptpp:
                    # first adj chunks + their masks run pre-gate:
                    # DMA early, binarize on the (idle) GPSIMD
                    early = {}
                    for p_ in sorted(POOL_BIN):
                        blk = BLK_ORDER[p_]
                        at = adjp.tile([128, 2, IR], f32, tag="adj")
                        nc.sync.dma_start(at[:], adjT_r[:, 2 * blk:2 * blk + 2, :])
                        mt = mtp.tile([128, 2, IR], f16, tag="mi")
                        nc.gpsimd.tensor_scalar(
                            mt[:], at[:], 0.5, None, op0=Alu.is_gt)
                        early[blk] = (at, mt)
                    gate_sigmoid()
                    pend = []
                    for bpos, blk in enumerate(BLK_ORDER):
                        if blk in early:
                            at, mt = early[blk]
                        else:
                            at = adjp.tile([128, 2, IR], f32, tag="adj")
                            nc.sync.dma_start(at[:],
                                              adjT_r[:, 2 * blk:2 * blk + 2, :])
                            mt = mtp.tile([128, 2, IR], f16, tag="mi")
                            if bpos in DVE_BIN:
                                nc.vector.tensor_scalar(
                                    mt[:], at[:], 0.5, None, op0=Alu.is_gt)
                            else:
                                nc.scalar.activation(
                                    mt[:].rearrange("p a b -> p (a b)"),
                                    at[:].rearrange("p a b -> p (a b)"),
                                    Act.Sigmoid, bias=sigB[:, 0:1], scale=1e6)
                        for q in range(2):
                            jt = 2 * blk + q
                            pos = 2 * bpos + q
                            nP = POOLH_SEQ[pos]
                            nD = 4 - nP
                            # pool-side first so GPSIMD starts early
                            if nP:
                                qP = qpp.tile([128, 2, IR], f16, tag="qP")
                                for h in range(nD, 4):
                                    nc.vector.tensor_scalar(
                                        qP[:, h - nD, :], wb16[:, h, :],
                                        tv1[:, jt, h:h + 1], tv2[:, jt, h:h + 1],
                                        op0=Alu.mult, op1=Alu.max)
                                ptP = ptpp.tile([128, 2, IR], f16, tag="ptP")
                                nc.gpsimd.tensor_tensor(
                                    ptP[:, 0:nP, :], qP[:, 0:nP, :],
                                    mt[:, q:q + 1, :].broadcast_to([128, nP, IR]),
                                    op=Alu.mult)
                            else:
                                ptP = None
                            relu01 = pos in RELU_POS
                            if relu01:
                                for h in (0, 1):
                                    nc.vector.tensor_scalar(
                                        g2pP[:, jt, 64 * h:64 * h + 33],
                                        Vpack[:, jt, h, :],
                                        tv2[:, jt, h:h + 1], None, op0=Alu.mult)
                            qD = qdp.tile([128, 4, IR], f16, tag="qD")
                            for h in range(nD):
                                if relu01 and h < 2:
                                    nc.scalar.activation(
                                        qD[:, h, :], wb16[:, h, :], Act.Relu,
                                        scale=tv1[:, jt, h:h + 1],
                                        bias=tv2n[:, jt, h:h + 1])
                                else:
                                    nc.vector.tensor_scalar(
                                        qD[:, h, :], wb16[:, h, :],
                                        tv1[:, jt, h:h + 1], tv2[:, jt, h:h + 1],
                                        op0=Alu.mult, op1=Alu.max)
                            ptD = ptdp.tile([128, 4, IR], f16, tag="ptD")
                            nc.vector.tensor_tensor(
                                ptD[:, 0:nD, :], qD[:, 0:nD, :],
                                mt[:, q:q + 1, :].broadcast_to([128, nD, IR]),
                                op=Alu.mult)
                            pend.append((jt, pos, nD, ptD, ptP, relu01, mt, q))
                        if bpos == 6:
                            ID33_dma()
                        # 32-matmul PE clusters (every 2 blocks) keep the PE
                        # p-state ramped; the last block goes alone, DVE-fed
                        # matmuls first and h0/h1 before h2/h3, so the
                        # finalization overlaps the trailing mask work
                        if bpos % 2 == 1 or bpos >= 6:
                            if bpos == 7:
                                order = sorted(
                                    ((h, e) for h in range(4) for e in pend),
                                    key=lambda he: (he[0] >= 4 - he[1][2], he[0]))
                            else:
                                order = [(h, e) for e in pend for h in range(4)]
                            mm_done = set()
                            for h, (jt, pos, nD, ptD, ptP, relu01, mte, qe) in order:
                                if relu01 and jt not in mm_done:
                                    # v2-part of heads 0/1 via mask-matmul
                                    mm_done.add(jt)
                                    for k in range(2):
                                        nc.tensor.matmul(
                                            pgA[:, k * 512:(k + 1) * 512],
                                            g2pP[:, jt, :],
                                            mte[:, qe, k * 512:(k + 1) * 512],
                                            start=False, stop=False)
                                if h < nD:
                                    pair_mm(h, jt, ptD, h, pos)
                                else:
                                    pair_mm(h, jt, ptP, h - nD, pos)
                            pend = []

                # finalize
                ftp_ctx = tc.tile_pool(name="ftp", bufs=2, space="PSUM")
                ftp = ftp_ctx.__enter__()
                ndp_ctx = tc.tile_pool(name="ndp", bufs=4)
                ndp = ndp_ctx.__enter__()
                try:
                    for h in range(4):
                        tile_, off = pgt[h]
                        numD = ndp.tile([33, IR], f16, tag="numD")
                        nc.scalar.copy(numD[:], tile_[off:off + 33, :])
                        tpA = ftp.tile([128, 8, 33], f32, tag="tpA")
                        for c in range(8):
                            nc.tensor.matmul(tpA[:, c, :],
                                             numD[:, c * 128:(c + 1) * 128],
                                             id33[:], start=True, stop=False)
                            nc.tensor.matmul(tpA[:, c, :],
                                             numD[32:33, c * 128:(c + 1) * 128],
                                             biasTE[32:33, h, :],
                                             start=False, stop=True)
                        rdT = ndp.tile([128, 8, 1], f32, tag="rdT")
                        nc.vector.reciprocal_approx_fast(rdT[:], tpA[:, :, 32:33])
                        nc.vector.tensor_tensor(
                            out_sb[:, :, h * 32:(h + 1) * 32], tpA[:, :, 0:32],
                            rdT[:, :, 0:1].broadcast_to([128, 8, 32]), op=Alu.mult)
                    nc.sync.dma_start(
                        out_d[:].rearrange("(s p) f -> p s f", p=128), out_sb[:])
                finally:
                    ndp_ctx.__exit__(None, None, None)
                    ftp_ctx.__exit__(None, None, None)
                    psg_ctx.__exit__(None, None, None)

            for _rep in range(reps):
                emit_body()
            pp_ctx.__exit__(None, None, None)
        finally:
            cst_ctx.__exit__(None, None, None)

    nc.compile()
    return nc


def _prepare_in_maps(x, adj, W, a_src, a_dst, bias):
    x = np.ascontiguousarray(np.asarray(x, dtype=np.float32))
    adj = np.asarray(adj, dtype=np.float32)
    W = np.asarray(W, dtype=np.float32)
    a_src = np.asarray(a_src, dtype=np.float32)
    a_dst = np.asarray(a_dst, dtype=np.float32)
    bias = np.asarray(bias, dtype=np.float32)

    Wf = np.ascontiguousarray(W.reshape(D, HF)) * 0.0625
    WaS = np.ascontiguousarray(np.einsum("dhf,hf->dh", W.reshape(D, H, F), a_src))
    WaD = np.ascontiguousarray(np.einsum("dhf,hf->dh", W.reshape(D, H, F), a_dst))
    biasRh = np.ascontiguousarray(bias.reshape(1, HF))

    in_maps = []
    for c in range(NCORES):
        b, cc = c // 2, c % 2
        i0 = cc * IR
        in_maps.append({
            "xT": np.ascontiguousarray(x[b].T),
            "xiT": np.ascontiguousarray(x[b, i0:i0 + IR].T),
            "adjT": np.ascontiguousarray(adj[b, i0:i0 + IR, :].T),
            "Wfs": Wf,
            "WaS": WaS,
            "WaD": WaD,
            "biasR": biasRh,
        })
    return in_maps


def run(inputs, trace=False, trace_cores=None):
    from concourse.bass_utils import run_bass_kernel_spmd
    if "nc" not in _CACHE:
        _CACHE["nc"] = build_nc()
    nc = _CACHE["nc"]
    in_maps = _prepare_in_maps(**inputs)
    kw = {}
    if trace:
        kw = dict(trace=True, trace_cores=trace_cores or [0])
    res = run_bass_kernel_spmd(nc, in_maps, list(range(NCORES)), **kw)
    out = np.zeros((B, N, HF), np.float32)
    for c in range(NCORES):
        b, cc = c // 2, c % 2
        out[b, cc * IR:(cc + 1) * IR, :] = res.results[c]["out"]
    return out, res


def kernel(**inputs):
    out, _ = run(inputs, trace=False)
    return out


# revision 78
# speedup vs baseline: 1.4075x; 1.0534x over previous
"""Batched GAT kernel for 8 Trainium2 NeuronCores.

Math: out[b,i,:] = softmax_j(mask(leakyrelu(s_i+t_j))) @ h  per head, concat heads.

Decomposition (per head): exp(lrelu(e)) / exp(.2 s_i) = max(w_i v1_j, v2_j)
with w = exp(.8 s), v1 = exp(t), v2 = exp(.2 t); the 1/exp(.2 s_i) row scale
cancels in the softmax ratio.

Layout: adj is HOST-transposed per core (adjT[j, i]), so the binarized mask is
born in the [j-partition, i-free] orientation the attention tiles use — no PE
transposes, no mask copies.  Per j-tile:
  q_h = tensor_scalar(wb_h, *v1_jh, max v2_jh)   (DVE, 4x mode)
  pt  = q * m  — split between one multi-head broadcast TT on DVE (2x mode)
        and a GPSIMD TT for 0-2 heads per tile (POOLH load-balance knob)
  pair matmuls vs Vpack = 0.0625*[h|1] accumulate num/den in PSUM, emitted in
  16-matmul per-block clusters to keep the PE p-state ramped.
s/t scores come from host-folded WaS = W@a_src, WaD = W@a_dst ([D,H]); the x
feature stream is processed in 512-column chunks interleaved with the adj
stream so first-tile compute starts ~6us in.  Finalize: transpose via 33x33
identity matmuls, reciprocal_approx_fast, fused bias matmul, single 512B-row
output DMA.
"""
import os
import sys
import numpy as np

for _p in ("/opt/trn_rl_repo",):
    if _p not in sys.path:
        sys.path.insert(0, _p)

B, N, D, H, F = 4, 2048, 128, 4, 32
HF = H * F           # 128
IR = 1024            # i-rows per core
NJT = N // 128       # 16 j-tiles
NCORES = 8

import json as _json

def _knob(name, default):
    v = os.environ.get(name)
    return _json.loads(v) if v else default

# adj blocks are processed in this order; the accumulation group opens on the
# first entry's j-tiles and closes on the last (process order is free since
# the PSUM accumulate is a sum)
BLK_ORDER = _knob("K_BORD", [7, 0, 1, 2, 3, 4, 5, 6])
# pool-head count per position in BLK_ORDER (load balance DVE vs GPSIMD;
# last positions 0 so GPSIMD drains before the finale)
POOLH_SEQ = _knob("K_PSEQ", [1, 1, 2, 1, 2, 1, 2, 1, 1, 2, 1, 1, 1, 1, 1, 1])
# blocks (by position) whose mask is binarized on GPSIMD (idle during
# startup) instead of waiting for the Act sigmoid table; DVE_BIN likewise on
# the vector engine (fills its pre-steady-state stall)
POOL_BIN = set(_knob("K_PBIN", [0, 1]))
DVE_BIN = set(_knob("K_DBIN", []))
# positions where heads 0/1 use the Act-engine relu route:
# p~ = v2 + relu(v1*w - v2); the relu part goes through the normal masked
# pair matmuls and the v2 part is recovered by a mask-matmul against the
# v2-scaled feature pack (same PSUM accumulation group)
RELU_POS = set(_knob("K_RELU", [7, 8, 9, 10, 11, 12, 13]))
BUFS = _knob("K_BUFS", {})
BUFS = {"adjp": 4, "mtp": 5, "qdp": 3, "qpp": 4, "ptdp": 4, "ptpp": 4, **BUFS}

_CACHE = {}


def build_nc(reps=1):
    import concourse.bacc as bacc
    import concourse.tile as tile
    from concourse import mybir

    f32, f16 = mybir.dt.float32, mybir.dt.float16
    Alu = mybir.AluOpType
    Act = mybir.ActivationFunctionType

    nc = bacc.Bacc(None, target_bir_lowering=False)

    xT_d   = nc.dram_tensor("xT",   [D, N],    f32, kind="ExternalInput")
    xiT_d  = nc.dram_tensor("xiT",  [D, IR],   f32, kind="ExternalInput")
    adjT_d = nc.dram_tensor("adjT", [N, IR],   f32, kind="ExternalInput")
    Wf_d   = nc.dram_tensor("Wfs",  [D, HF],   f32, kind="ExternalInput")
    WaS_d  = nc.dram_tensor("WaS",  [D, H],    f32, kind="ExternalInput")
    WaD_d  = nc.dram_tensor("WaD",  [D, H],    f32, kind="ExternalInput")
    bias_d = nc.dram_tensor("biasR", [1, HF],  f32, kind="ExternalInput")
    out_d  = nc.dram_tensor("out",  [IR, HF],  f32, kind="ExternalOutput")

    EY = np.zeros((4, 4 * 128), np.float16)
    for h in range(H):
        EY[h, h * 128:(h + 1) * 128] = 1.0
    EY_d = nc.inline_tensor(EY, "EYc")
    ID33_d = nc.inline_tensor(np.eye(33, dtype=np.float16), "id33c")

    adjT_r = adjT_d[:].rearrange("(s p) i -> p s i", p=128)

    with tile.TileContext(nc) as tc:
        cst_ctx = tc.tile_pool(name="cst", bufs=1)
        cst = cst_ctx.__enter__()
        try:
            xiT   = cst.tile([D, IR], f32)
            xT    = cst.tile([D, N], f32)
            Wf    = cst.tile([D, HF], f32)
            WaS   = cst.tile([D, H], f32)
            WaD   = cst.tile([D, H], f32)
            biasR = cst.tile([1, HF], f32)
            biasTE = cst.tile([64, 4, 33], f16)
            eyc   = cst.tile([4, 4 * 128], f16)
            id33  = cst.tile([33, 33], f16)
            sigB  = cst.tile([128, 1], f32)

            Wf16  = cst.tile([D, HF], f16)
            WaS16 = cst.tile([D, H], f16)
            WaD16 = cst.tile([D, H], f16)
            xT16  = cst.tile([D, N], f16)
            xiT16 = cst.tile([D, IR], f16)
            sZ4   = cst.tile([4, IR], f32)
            w16   = cst.tile([4, IR], f16)          # exp(.8 s)
            wb16  = cst.tile([128, H, IR], f16)     # per-head broadcast of w16
            tAll  = cst.tile([128, NJT, H], f32)
            tv1   = cst.tile([128, NJT, H], f32)    # exp(t)
            tv2   = cst.tile([128, NJT, H], f32)    # exp(.2 t)
            tv2n  = cst.tile([128, NJT, H], f32)    # -exp(.2 t)
            Vpack = cst.tile([128, NJT, H, 33], f16)
            g2pP  = cst.tile([128, NJT, 97], f16)   # v2-scaled packs, pg rows
            out_sb = cst.tile([128, 8, HF], f32)

            # -------- DMA order: score path first, adj stream next --------
            nc.sync.dma_start(xiT[:], xiT_d[:])
            nc.sync.dma_start(WaS[:], WaS_d[:])
            # (adjT chunk DMAs are issued inside the block loop; pool bufs=3
            # lets the first three queue right behind these)
            nc.sync.dma_start(Wf[:], Wf_d[:])
            nc.sync.dma_start(WaD[:], WaD_d[:])
            nc.sync.dma_start(eyc[:], EY_d[:])
            nc.vector.memset(biasTE[:], 0.0)
            nc.gpsimd.memset(Vpack[:, :, :, 32:33], 0.0625)

            # -------- score/weight prep (DVE does the copies: it is idle
            # during startup and the bottleneck later) --------
            nc.vector.tensor_copy(WaS16[:], WaS[:])
            nc.vector.tensor_copy(WaD16[:], WaD[:])
            nc.vector.tensor_copy(Wf16[:], Wf[:])
            for k in range(2):
                nc.vector.tensor_copy(xiT16[:, k * 512:(k + 1) * 512],
                                      xiT[:, k * 512:(k + 1) * 512])

            pp_ctx = tc.tile_pool(name="pp", bufs=2, space="PSUM")
            pp = pp_ctx.__enter__()
            for k in range(2):
                ps = pp.tile([4, 512], f32, tag="pp")
                nc.tensor.matmul(ps[:], WaS16[:], xiT16[:, k * 512:(k + 1) * 512],
                                 start=True, stop=True)
                nc.vector.tensor_copy(sZ4[:, k * 512:(k + 1) * 512], ps[:])
            nc.scalar.activation(w16[:], sZ4[:], Act.Exp, scale=0.8)
            # w broadcasts immediately (DVE is idle now; h3 first since the
            # pool-side TS ops consume it soonest)
            for h in (3, 2, 1, 0):
                for k in range(2):
                    ps = pp.tile([128, 512], f32, tag="pp")
                    nc.tensor.matmul(ps[:], eyc[:, h * 128:(h + 1) * 128],
                                     w16[:, k * 512:(k + 1) * 512],
                                     start=True, stop=True)
                    nc.vector.tensor_copy(wb16[:, h, k * 512:(k + 1) * 512],
                                          ps[:])

            def xchunk_scores(g):
                # x columns [512g, 512(g+1)): t-scores + exps (gates sigmoids)
                nc.sync.dma_start(xT[:, g * 512:(g + 1) * 512],
                                  xT_d[:, g * 512:(g + 1) * 512])
                nc.scalar.copy(xT16[:, g * 512:(g + 1) * 512],
                               xT[:, g * 512:(g + 1) * 512])
                ps = pp.tile([128, 4 * H], f32, tag="pp")
                for k4 in range(4):
                    jt = g * 4 + k4
                    nc.tensor.matmul(ps[:, k4 * H:(k4 + 1) * H],
                                     xT16[:, jt * 128:(jt + 1) * 128], WaD16[:],
                                     start=True, stop=True)
                nc.scalar.copy(tAll[:, g * 4:(g + 1) * 4, :], ps[:])
                nc.scalar.activation(
                    tv1[:, g * 4:(g + 1) * 4, :].rearrange("p a b -> p (a b)"),
                    tAll[:, g * 4:(g + 1) * 4, :].rearrange("p a b -> p (a b)"),
                    Act.Exp)
                nc.scalar.activation(
                    tv2[:, g * 4:(g + 1) * 4, :].rearrange("p a b -> p (a b)"),
                    tAll[:, g * 4:(g + 1) * 4, :].rearrange("p a b -> p (a b)"),
                    Act.Exp, scale=0.2)

            def xchunk_vpack(blk):
                jt0 = 2 * blk
                pv = pp.tile([128, 256], f32, tag="pp")
                for d in range(2):
                    nc.tensor.matmul(
                        pv[:, d * 128:(d + 1) * 128],
                        xT16[:, (jt0 + d) * 128:(jt0 + d + 1) * 128],
                        Wf16[:], start=True, stop=True)
                nc.scalar.copy(
                    Vpack[:, jt0:jt0 + 2, :, 0:32],
                    pv[:].rearrange("p (j h f) -> p j h f", j=2, h=H))

            # late, fin-only params
            ID33_dma = lambda: (nc.sync.dma_start(id33[:], ID33_d[:]),
                                nc.sync.dma_start(biasR[:], bias_d[:]),
                                nc.scalar.copy(
                                    biasTE[32:33, :, 0:32],
                                    biasR[:].rearrange("p (h f) -> p h f", h=H)))

            def gate_sigmoid():
                # sigB is the sigmoid bias; producing it from the last exp
                # output (a Copy, so no table change) forces every Exp before
                # the first Sigmoid -> exactly one exp->sigmoid table switch.
                # g=3 first: BLK_ORDER starts at block 7 whose t-scores need
                # the last x column chunk.
                for g in (3, 0, 1, 2):
                    xchunk_scores(g)
                nc.scalar.activation(sigB[:], tv2[:, 11, 3:4], Act.Copy,
                                     bias=-5e5, scale=0.0)
                nc.vector.tensor_scalar(
                    tv2n[:].rearrange("p a b -> p (a b)"),
                    tv2[:].rearrange("p a b -> p (a b)"), -1.0, None,
                    op0=Alu.mult)
                if RELU_POS:
                    nc.vector.memset(g2pP[:, :, 33:64], 0.0)
                for blk in BLK_ORDER:
                    xchunk_vpack(blk)

            # ---------------- main body (per rep) ----------------
            def emit_body():
                psg_ctx = tc.tile_pool(name="psg", bufs=1, space="PSUM")
                psg = psg_ctx.__enter__()
                pgA = psg.tile([97, IR], f32, tag="pgA", name="pgA")
                pgB = psg.tile([97, IR], f32, tag="pgB", name="pgB")
                pgt = {0: (pgA, 0), 1: (pgA, 64), 2: (pgB, 0), 3: (pgB, 64)}

                def pair_mm(h, jt, pt, hh, pos):
                    tile_, off = pgt[h]
                    for k in range(2):
                        nc.tensor.matmul(
                            tile_[off:off + 33, k * 512:(k + 1) * 512],
                            Vpack[:, jt, h, :],
                            pt[:, hh, k * 512:(k + 1) * 512],
                            start=(pos == 0), stop=(pos == NJT - 1))

                with tc.tile_pool(name="adjp", bufs=BUFS["adjp"]) as adjp, \
                     tc.tile_pool(name="mtp", bufs=BUFS["mtp"]) as mtp, \
                     tc.tile_pool(name="qdp", bufs=BUFS["qdp"]) as qdp, \
                     tc.tile_pool(name="qpp", bufs=BUFS["qpp"]) as qpp, \
                     tc.tile_pool(name="ptdp", bufs=BUFS["ptdp"]) as ptdp, \
                     tc.tile_pool(name="ptpp", bufs=BUFS["ptpp"]) as ptpp:
                    # first adj chunks + their masks run pre-gate:
                    # DMA early, binarize on the (idle) GPSIMD
                    early = {}
                    for p_ in sorted(p for p in POOL_BIN if p < 2):
                        blk = BLK_ORDER[p_]
                        at = adjp.tile([128, 2, IR], f32, tag="adj")
                        nc.sync.dma_start(at[:], adjT_r[:, 2 * blk:2 * blk + 2, :])
                        mt = mtp.tile([128, 2, IR], f16, tag="mi")
                        nc.gpsimd.tensor_scalar(
                            mt[:], at[:], 0.5, None, op0=Alu.is_gt)
                        early[blk] = (at, mt)
                    gate_sigmoid()
                    pend = []
                    for bpos, blk in enumerate(BLK_ORDER):
                        if blk in early:
                            at, mt = early[blk]
                        else:
                            at = adjp.tile([128, 2, IR], f32, tag="adj")
                            nc.sync.dma_start(at[:],
                                              adjT_r[:, 2 * blk:2 * blk + 2, :])
                            mt = mtp.tile([128, 2, IR], f16, tag="mi")
                            if bpos in DVE_BIN:
                                nc.vector.tensor_scalar(
                                    mt[:], at[:], 0.5, None, op0=Alu.is_gt)
                            elif bpos in POOL_BIN:
                                nc.gpsimd.tensor_scalar(
                                    mt[:], at[:], 0.5, None, op0=Alu.is_gt)
                            else:
                                nc.scalar.activation(
                                    mt[:].rearrange("p a b -> p (a b)"),
                                    at[:].rearrange("p a b -> p (a b)"),
                                    Act.Sigmoid, bias=sigB[:, 0:1], scale=1e6)
                        for q in range(2):
                            jt = 2 * blk + q
                            pos = 2 * bpos + q
                            nP = POOLH_SEQ[pos]
                            nD = 4 - nP
                            # pool-side first so GPSIMD starts early
                            if nP:
                                qP = qpp.tile([128, 2, IR], f16, tag="qP")
                                for h in range(nD, 4):
                                    nc.vector.tensor_scalar(
                                        qP[:, h - nD, :], wb16[:, h, :],
                                        tv1[:, jt, h:h + 1], tv2[:, jt, h:h + 1],
                                        op0=Alu.mult, op1=Alu.max)
                                ptP = ptpp.tile([128, 2, IR], f16, tag="ptP")
                                nc.gpsimd.tensor_tensor(
                                    ptP[:, 0:nP, :], qP[:, 0:nP, :],
                                    mt[:, q:q + 1, :].broadcast_to([128, nP, IR]),
                                    op=Alu.mult)
                            else:
                                ptP = None
                            relu01 = pos in RELU_POS
                            if relu01:
                                for h in (0, 1):
                                    nc.vector.tensor_scalar(
                                        g2pP[:, jt, 64 * h:64 * h + 33],
                                        Vpack[:, jt, h, :],
                                        tv2[:, jt, h:h + 1], None, op0=Alu.mult)
                            qD = qdp.tile([128, 4, IR], f16, tag="qD")
                            for h in range(nD):
                                if relu01 and h < 2:
                                    nc.scalar.activation(
                                        qD[:, h, :], wb16[:, h, :], Act.Relu,
                                        scale=tv1[:, jt, h:h + 1],
                                        bias=tv2n[:, jt, h:h + 1])
                                else:
                                    nc.vector.tensor_scalar(
                                        qD[:, h, :], wb16[:, h, :],
                                        tv1[:, jt, h:h + 1], tv2[:, jt, h:h + 1],
                                        op0=Alu.mult, op1=Alu.max)
                            ptD = ptdp.tile([128, 4, IR], f16, tag="ptD")
                            nc.vector.tensor_tensor(
                                ptD[:, 0:nD, :], qD[:, 0:nD, :],
                                mt[:, q:q + 1, :].broadcast_to([128, nD, IR]),
                                op=Alu.mult)
                            pend.append((jt, pos, nD, ptD, ptP, relu01, mt, q))
                        if bpos == 6:
                            ID33_dma()
                        # 32-matmul PE clusters (every 2 blocks) keep the PE
                        # p-state ramped; the last block goes alone, DVE-fed
                        # matmuls first and h0/h1 before h2/h3, so the
                        # finalization overlaps the trailing mask work
                        if bpos % 2 == 1 or bpos >= 6:
                            if bpos == 7:
                                order = sorted(
                                    ((h, e) for h in range(4) for e in pend),
                                    key=lambda he: (he[0] >= 4 - he[1][2], he[0]))
                            else:
                                order = [(h, e) for e in pend for h in range(4)]
                            mm_done = set()
                            for h, (jt, pos, nD, ptD, ptP, relu01, mte, qe) in order:
                                if relu01 and jt not in mm_done:
                                    # v2-part of heads 0/1 via mask-matmul
                                    mm_done.add(jt)
                                    for k in range(2):
                                        nc.tensor.matmul(
                                            pgA[:, k * 512:(k + 1) * 512],
                                            g2pP[:, jt, :],
                                            mte[:, qe, k * 512:(k + 1) * 512],
                                            start=False, stop=False)
                                if h < nD:
                                    pair_mm(h, jt, ptD, h, pos)
                                else:
                                    pair_mm(h, jt, ptP, h - nD, pos)
                            pend = []

                # finalize
                ftp_ctx = tc.tile_pool(name="ftp", bufs=2, space="PSUM")
                ftp = ftp_ctx.__enter__()
                ndp_ctx = tc.tile_pool(name="ndp", bufs=4)
                ndp = ndp_ctx.__enter__()
                try:
                    for h in range(4):
                        tile_, off = pgt[h]
                        numD = ndp.tile([33, IR], f16, tag="numD")
                        nc.scalar.copy(numD[:], tile_[off:off + 33, :])
                        tpA = ftp.tile([128, 8, 33], f32, tag="tpA")
                        for c in range(8):
                            nc.tensor.matmul(tpA[:, c, :],
                                             numD[:, c * 128:(c + 1) * 128],
                                             id33[:], start=True, stop=False)
                            nc.tensor.matmul(tpA[:, c, :],
                                             numD[32:33, c * 128:(c + 1) * 128],
                                             biasTE[32:33, h, :],
                                             start=False, stop=True)
                        rdT = ndp.tile([128, 8, 1], f32, tag="rdT")
                        nc.vector.reciprocal_approx_fast(rdT[:], tpA[:, :, 32:33])
                        nc.vector.tensor_tensor(
                            out_sb[:, :, h * 32:(h + 1) * 32], tpA[:, :, 0:32],
                            rdT[:, :, 0:1].broadcast_to([128, 8, 32]), op=Alu.mult)
                    nc.sync.dma_start(
                        out_d[:].rearrange("(s p) f -> p s f", p=128), out_sb[:])
                finally:
                    ndp_ctx.__exit__(None, None, None)
                    ftp_ctx.__exit__(None, None, None)
                    psg_ctx.__exit__(None, None, None)

            for _rep in range(reps):
                emit_body()
            pp_ctx.__exit__(None, None, None)
        finally:
            cst_ctx.__exit__(None, None, None)

    nc.compile()
    return nc


def _prepare_in_maps(x, adj, W, a_src, a_dst, bias):
    x = np.ascontiguousarray(np.asarray(x, dtype=np.float32))
    adj = np.asarray(adj, dtype=np.float32)
    W = np.asarray(W, dtype=np.float32)
    a_src = np.asarray(a_src, dtype=np.float32)
    a_dst = np.asarray(a_dst, dtype=np.float32)
    bias = np.asarray(bias, dtype=np.float32)

    Wf = np.ascontiguousarray(W.reshape(D, HF)) * 0.0625
    WaS = np.ascontiguousarray(np.einsum("dhf,hf->dh", W.reshape(D, H, F), a_src))
    WaD = np.ascontiguousarray(np.einsum("dhf,hf->dh", W.reshape(D, H, F), a_dst))
    biasRh = np.ascontiguousarray(bias.reshape(1, HF))

    in_maps = []
    for c in range(NCORES):
        b, cc = c // 2, c % 2
        i0 = cc * IR
        in_maps.append({
            "xT": np.ascontiguousarray(x[b].T),
            "xiT": np.ascontiguousarray(x[b, i0:i0 + IR].T),
            "adjT": np.ascontiguousarray(adj[b, i0:i0 + IR, :].T),
            "Wfs": Wf,
            "WaS": WaS,
            "WaD": WaD,
            "biasR": biasRh,
        })
    return in_maps


def run(inputs, trace=False, trace_cores=None):
    from concourse.bass_utils import run_bass_kernel_spmd
    if "nc" not in _CACHE:
        _CACHE["nc"] = build_nc()
    nc = _CACHE["nc"]
    in_maps = _prepare_in_maps(**inputs)
    kw = {}
    if trace:
        kw = dict(trace=True, trace_cores=trace_cores or [0])
    res = run_bass_kernel_spmd(nc, in_maps, list(range(NCORES)), **kw)
    out = np.zeros((B, N, HF), np.float32)
    for c in range(NCORES):
        b, cc = c // 2, c % 2
        out[b, cc * IR:(cc + 1) * IR, :] = res.results[c]["out"]
    return out, res


def kernel(**inputs):
    out, _ = run(inputs, trace=False)
    return out


# revision 83
# speedup vs baseline: 1.4191x; 1.0082x over previous
"""Batched GAT kernel for 8 Trainium2 NeuronCores.

Math: out[b,i,:] = softmax_j(mask(leakyrelu(s_i+t_j))) @ h  per head, concat heads.

Decomposition (per head): exp(lrelu(e)) / exp(.2 s_i) = max(w_i v1_j, v2_j)
with w = exp(.8 s), v1 = exp(t), v2 = exp(.2 t); the 1/exp(.2 s_i) row scale
cancels in the softmax ratio.

Layout: adj is HOST-transposed per core (adjT[j, i]), so the binarized mask is
born in the [j-partition, i-free] orientation the attention tiles use — no PE
transposes, no mask copies.  Per j-tile:
  q_h = tensor_scalar(wb_h, *v1_jh, max v2_jh)   (DVE, 4x mode)
  pt  = q * m  — split between one multi-head broadcast TT on DVE (2x mode)
        and a GPSIMD TT for 0-2 heads per tile (POOLH load-balance knob)
  pair matmuls vs Vpack = 0.0625*[h|1] accumulate num/den in PSUM, emitted in
  16-matmul per-block clusters to keep the PE p-state ramped.
s/t scores come from host-folded WaS = W@a_src, WaD = W@a_dst ([D,H]); the x
feature stream is processed in 512-column chunks interleaved with the adj
stream so first-tile compute starts ~6us in.  Finalize: transpose via 33x33
identity matmuls, reciprocal_approx_fast, fused bias matmul, single 512B-row
output DMA.
"""
import os
import sys
import numpy as np

for _p in ("/opt/trn_rl_repo",):
    if _p not in sys.path:
        sys.path.insert(0, _p)

B, N, D, H, F = 4, 2048, 128, 4, 32
HF = H * F           # 128
IR = 1024            # i-rows per core
NJT = N // 128       # 16 j-tiles
NCORES = 8

import json as _json

def _knob(name, default):
    v = os.environ.get(name)
    return _json.loads(v) if v else default

# adj blocks are processed in this order; the accumulation group opens on the
# first entry's j-tiles and closes on the last (process order is free since
# the PSUM accumulate is a sum)
BLK_ORDER = _knob("K_BORD", [7, 0, 1, 2, 3, 4, 5, 6])
# pool-head count per position in BLK_ORDER (load balance DVE vs GPSIMD;
# last positions 0 so GPSIMD drains before the finale)
POOLH_SEQ = _knob("K_PSEQ", [1, 2, 1, 2, 1, 2, 1, 1, 2, 1, 1, 1, 1, 1, 1, 1])
# blocks (by position) whose mask is binarized on GPSIMD (idle during
# startup) instead of waiting for the Act sigmoid table; DVE_BIN likewise on
# the vector engine (fills its pre-steady-state stall)
POOL_BIN = set(_knob("K_PBIN", [0, 1]))
DVE_BIN = set(_knob("K_DBIN", []))
# positions where heads 0/1 use the Act-engine relu route:
# p~ = v2 + relu(v1*w - v2); the relu part goes through the normal masked
# pair matmuls and the v2 part is recovered by a mask-matmul against the
# v2-scaled feature pack (same PSUM accumulation group)
RELU_POS = set(_knob("K_RELU", [6, 7, 8, 9, 10, 11, 12, 13]))
FINND = _knob("K_FINND", "AAAA")
BUFS = _knob("K_BUFS", {})
BUFS = {"adjp": 4, "mtp": 5, "qdp": 3, "qpp": 4, "ptdp": 4, "ptpp": 4, **BUFS}

_CACHE = {}


def build_nc(reps=1):
    import concourse.bacc as bacc
    import concourse.tile as tile
    from concourse import mybir

    f32, f16 = mybir.dt.float32, mybir.dt.float16
    Alu = mybir.AluOpType
    Act = mybir.ActivationFunctionType

    nc = bacc.Bacc(None, target_bir_lowering=False)

    xT_d   = nc.dram_tensor("xT",   [D, N],    f32, kind="ExternalInput")
    xiT_d  = nc.dram_tensor("xiT",  [D, IR],   f32, kind="ExternalInput")
    adjT_d = nc.dram_tensor("adjT", [N, IR],   f32, kind="ExternalInput")
    Wf_d   = nc.dram_tensor("Wfs",  [D, HF],   f32, kind="ExternalInput")
    WaS_d  = nc.dram_tensor("WaS",  [D, H],    f32, kind="ExternalInput")
    WaD_d  = nc.dram_tensor("WaD",  [D, H],    f32, kind="ExternalInput")
    bias_d = nc.dram_tensor("biasR", [1, HF],  f32, kind="ExternalInput")
    out_d  = nc.dram_tensor("out",  [IR, HF],  f32, kind="ExternalOutput")

    EY = np.zeros((4, 4 * 128), np.float16)
    for h in range(H):
        EY[h, h * 128:(h + 1) * 128] = 1.0
    EY_d = nc.inline_tensor(EY, "EYc")
    ID33_d = nc.inline_tensor(np.eye(33, dtype=np.float16), "id33c")

    adjT_r = adjT_d[:].rearrange("(s p) i -> p s i", p=128)

    with tile.TileContext(nc) as tc:
        cst_ctx = tc.tile_pool(name="cst", bufs=1)
        cst = cst_ctx.__enter__()
        try:
            xiT   = cst.tile([D, IR], f32)
            xT    = cst.tile([D, N], f32)
            Wf    = cst.tile([D, HF], f32)
            WaS   = cst.tile([D, H], f32)
            WaD   = cst.tile([D, H], f32)
            biasR = cst.tile([1, HF], f32)
            biasTE = cst.tile([64, 4, 33], f16)
            eyc   = cst.tile([4, 4 * 128], f16)
            id33  = cst.tile([33, 33], f16)
            sigB  = cst.tile([128, 1], f32)

            Wf16  = cst.tile([D, HF], f16)
            WaS16 = cst.tile([D, H], f16)
            WaD16 = cst.tile([D, H], f16)
            xT16  = cst.tile([D, N], f16)
            xiT16 = cst.tile([D, IR], f16)
            sZ4   = cst.tile([4, IR], f32)
            w16   = cst.tile([4, IR], f16)          # exp(.8 s)
            wb16  = cst.tile([128, H, IR], f16)     # per-head broadcast of w16
            tAll  = cst.tile([128, NJT, H], f32)
            tv1   = cst.tile([128, NJT, H], f32)    # exp(t)
            tv2   = cst.tile([128, NJT, H], f32)    # exp(.2 t)
            tv2n  = cst.tile([128, NJT, H], f32)    # -exp(.2 t)
            Vpack = cst.tile([128, NJT, H, 33], f16)
            g2pP  = cst.tile([128, NJT, 97], f16)   # v2-scaled packs, pg rows
            out_sb = cst.tile([128, 8, HF], f32)

            # -------- DMA order: score path first, adj stream next --------
            nc.sync.dma_start(xiT[:], xiT_d[:])
            nc.sync.dma_start(WaS[:], WaS_d[:])
            # (adjT chunk DMAs are issued inside the block loop; pool bufs=3
            # lets the first three queue right behind these)
            nc.sync.dma_start(Wf[:], Wf_d[:])
            nc.sync.dma_start(WaD[:], WaD_d[:])
            nc.sync.dma_start(eyc[:], EY_d[:])
            nc.vector.memset(biasTE[:], 0.0)
            nc.gpsimd.memset(Vpack[:, :, :, 32:33], 0.0625)

            # -------- score/weight prep (DVE does the copies: it is idle
            # during startup and the bottleneck later) --------
            nc.vector.tensor_copy(WaS16[:], WaS[:])
            nc.vector.tensor_copy(WaD16[:], WaD[:])
            nc.vector.tensor_copy(Wf16[:], Wf[:])
            for k in range(2):
                nc.vector.tensor_copy(xiT16[:, k * 512:(k + 1) * 512],
                                      xiT[:, k * 512:(k + 1) * 512])

            pp_ctx = tc.tile_pool(name="pp", bufs=2, space="PSUM")
            pp = pp_ctx.__enter__()
            for k in range(2):
                ps = pp.tile([4, 512], f32, tag="pp")
                nc.tensor.matmul(ps[:], WaS16[:], xiT16[:, k * 512:(k + 1) * 512],
                                 start=True, stop=True)
                nc.vector.tensor_copy(sZ4[:, k * 512:(k + 1) * 512], ps[:])
            nc.scalar.activation(w16[:], sZ4[:], Act.Exp, scale=0.8)
            # w broadcasts immediately (DVE is idle now; h3 first since the
            # pool-side TS ops consume it soonest)
            for h in (3, 2, 1, 0):
                for k in range(2):
                    ps = pp.tile([128, 512], f32, tag="pp")
                    nc.tensor.matmul(ps[:], eyc[:, h * 128:(h + 1) * 128],
                                     w16[:, k * 512:(k + 1) * 512],
                                     start=True, stop=True)
                    nc.vector.tensor_copy(wb16[:, h, k * 512:(k + 1) * 512],
                                          ps[:])

            def xchunk_scores(g):
                # x columns [512g, 512(g+1)): t-scores + exps (gates sigmoids)
                nc.sync.dma_start(xT[:, g * 512:(g + 1) * 512],
                                  xT_d[:, g * 512:(g + 1) * 512])
                nc.scalar.copy(xT16[:, g * 512:(g + 1) * 512],
                               xT[:, g * 512:(g + 1) * 512])
                ps = pp.tile([128, 4 * H], f32, tag="pp")
                for k4 in range(4):
                    jt = g * 4 + k4
                    nc.tensor.matmul(ps[:, k4 * H:(k4 + 1) * H],
                                     xT16[:, jt * 128:(jt + 1) * 128], WaD16[:],
                                     start=True, stop=True)
                nc.scalar.copy(tAll[:, g * 4:(g + 1) * 4, :], ps[:])
                nc.scalar.activation(
                    tv1[:, g * 4:(g + 1) * 4, :].rearrange("p a b -> p (a b)"),
                    tAll[:, g * 4:(g + 1) * 4, :].rearrange("p a b -> p (a b)"),
                    Act.Exp)
                nc.scalar.activation(
                    tv2[:, g * 4:(g + 1) * 4, :].rearrange("p a b -> p (a b)"),
                    tAll[:, g * 4:(g + 1) * 4, :].rearrange("p a b -> p (a b)"),
                    Act.Exp, scale=0.2)

            def xchunk_vpack(blk):
                jt0 = 2 * blk
                pv = pp.tile([128, 256], f32, tag="pp")
                for d in range(2):
                    nc.tensor.matmul(
                        pv[:, d * 128:(d + 1) * 128],
                        xT16[:, (jt0 + d) * 128:(jt0 + d + 1) * 128],
                        Wf16[:], start=True, stop=True)
                nc.scalar.copy(
                    Vpack[:, jt0:jt0 + 2, :, 0:32],
                    pv[:].rearrange("p (j h f) -> p j h f", j=2, h=H))

            # late, fin-only params
            ID33_dma = lambda: (nc.sync.dma_start(id33[:], ID33_d[:]),
                                nc.sync.dma_start(biasR[:], bias_d[:]),
                                nc.scalar.copy(
                                    biasTE[32:33, :, 0:32],
                                    biasR[:].rearrange("p (h f) -> p h f", h=H)))

            def gate_sigmoid():
                # sigB is the sigmoid bias; producing it from the last exp
                # output (a Copy, so no table change) forces every Exp before
                # the first Sigmoid -> exactly one exp->sigmoid table switch.
                # g=3 first: BLK_ORDER starts at block 7 whose t-scores need
                # the last x column chunk.
                for g in (3, 0, 1, 2):
                    xchunk_scores(g)
                nc.scalar.activation(sigB[:], tv2[:, 11, 3:4], Act.Copy,
                                     bias=-5e5, scale=0.0)
                nc.vector.tensor_scalar(
                    tv2n[:].rearrange("p a b -> p (a b)"),
                    tv2[:].rearrange("p a b -> p (a b)"), -1.0, None,
                    op0=Alu.mult)
                if RELU_POS:
                    nc.vector.memset(g2pP[:, :, 33:64], 0.0)
                for blk in BLK_ORDER:
                    xchunk_vpack(blk)

            # ---------------- main body (per rep) ----------------
            def emit_body():
                psg_ctx = tc.tile_pool(name="psg", bufs=1, space="PSUM")
                psg = psg_ctx.__enter__()
                pgA = psg.tile([97, IR], f32, tag="pgA", name="pgA")
                pgB = psg.tile([97, IR], f32, tag="pgB", name="pgB")
                pgt = {0: (pgA, 0), 1: (pgA, 64), 2: (pgB, 0), 3: (pgB, 64)}

                def pair_mm(h, jt, pt, hh, pos):
                    tile_, off = pgt[h]
                    for k in range(2):
                        nc.tensor.matmul(
                            tile_[off:off + 33, k * 512:(k + 1) * 512],
                            Vpack[:, jt, h, :],
                            pt[:, hh, k * 512:(k + 1) * 512],
                            start=(pos == 0), stop=(pos == NJT - 1))

                with tc.tile_pool(name="adjp", bufs=BUFS["adjp"]) as adjp, \
                     tc.tile_pool(name="mtp", bufs=BUFS["mtp"]) as mtp, \
                     tc.tile_pool(name="qdp", bufs=BUFS["qdp"]) as qdp, \
                     tc.tile_pool(name="qpp", bufs=BUFS["qpp"]) as qpp, \
                     tc.tile_pool(name="ptdp", bufs=BUFS["ptdp"]) as ptdp, \
                     tc.tile_pool(name="ptpp", bufs=BUFS["ptpp"]) as ptpp:
                    # first adj chunks + their masks run pre-gate:
                    # DMA early, binarize on the (idle) GPSIMD
                    early = {}
                    for p_ in sorted(p for p in POOL_BIN if p < 2):
                        blk = BLK_ORDER[p_]
                        at = adjp.tile([128, 2, IR], f32, tag="adj")
                        nc.sync.dma_start(at[:], adjT_r[:, 2 * blk:2 * blk + 2, :])
                        mt = mtp.tile([128, 2, IR], f16, tag="mi")
                        nc.gpsimd.tensor_scalar(
                            mt[:], at[:], 0.5, None, op0=Alu.is_gt)
                        early[blk] = (at, mt)
                    gate_sigmoid()
                    pend = []
                    for bpos, blk in enumerate(BLK_ORDER):
                        if blk in early:
                            at, mt = early[blk]
                        else:
                            at = adjp.tile([128, 2, IR], f32, tag="adj")
                            nc.sync.dma_start(at[:],
                                              adjT_r[:, 2 * blk:2 * blk + 2, :])
                            mt = mtp.tile([128, 2, IR], f16, tag="mi")
                            if bpos in DVE_BIN:
                                nc.vector.tensor_scalar(
                                    mt[:], at[:], 0.5, None, op0=Alu.is_gt)
                            elif bpos in POOL_BIN:
                                nc.gpsimd.tensor_scalar(
                                    mt[:], at[:], 0.5, None, op0=Alu.is_gt)
                            else:
                                nc.scalar.activation(
                                    mt[:].rearrange("p a b -> p (a b)"),
                                    at[:].rearrange("p a b -> p (a b)"),
                                    Act.Sigmoid, bias=sigB[:, 0:1], scale=1e6)
                        for q in range(2):
                            jt = 2 * blk + q
                            pos = 2 * bpos + q
                            nP = POOLH_SEQ[pos]
                            nD = 4 - nP
                            # pool-side first so GPSIMD starts early
                            if nP:
                                qP = qpp.tile([128, 2, IR], f16, tag="qP")
                                for h in range(nD, 4):
                                    nc.vector.tensor_scalar(
                                        qP[:, h - nD, :], wb16[:, h, :],
                                        tv1[:, jt, h:h + 1], tv2[:, jt, h:h + 1],
                                        op0=Alu.mult, op1=Alu.max)
                                ptP = ptpp.tile([128, 2, IR], f16, tag="ptP")
                                nc.gpsimd.tensor_tensor(
                                    ptP[:, 0:nP, :], qP[:, 0:nP, :],
                                    mt[:, q:q + 1, :].broadcast_to([128, nP, IR]),
                                    op=Alu.mult)
                            else:
                                ptP = None
                            relu01 = pos in RELU_POS
                            if relu01:
                                for h in (0, 1):
                                    nc.vector.tensor_scalar(
                                        g2pP[:, jt, 64 * h:64 * h + 33],
                                        Vpack[:, jt, h, :],
                                        tv2[:, jt, h:h + 1], None, op0=Alu.mult)
                            qD = qdp.tile([128, 4, IR], f16, tag="qD")
                            for h in range(nD):
                                if relu01 and h < 2:
                                    nc.scalar.activation(
                                        qD[:, h, :], wb16[:, h, :], Act.Relu,
                                        scale=tv1[:, jt, h:h + 1],
                                        bias=tv2n[:, jt, h:h + 1])
                                else:
                                    nc.vector.tensor_scalar(
                                        qD[:, h, :], wb16[:, h, :],
                                        tv1[:, jt, h:h + 1], tv2[:, jt, h:h + 1],
                                        op0=Alu.mult, op1=Alu.max)
                            ptD = ptdp.tile([128, 4, IR], f16, tag="ptD")
                            nc.vector.tensor_tensor(
                                ptD[:, 0:nD, :], qD[:, 0:nD, :],
                                mt[:, q:q + 1, :].broadcast_to([128, nD, IR]),
                                op=Alu.mult)
                            pend.append((jt, pos, nD, ptD, ptP, relu01, mt, q))
                        if bpos == 6:
                            ID33_dma()
                        # 32-matmul PE clusters (every 2 blocks) keep the PE
                        # p-state ramped; the last block goes alone, DVE-fed
                        # matmuls first and h0/h1 before h2/h3, so the
                        # finalization overlaps the trailing mask work
                        if bpos % 2 == 1 or bpos >= 6:
                            if bpos == 7:
                                order = sorted(
                                    ((h, e) for h in range(4) for e in pend),
                                    key=lambda he: (he[0] >= 4 - he[1][2], he[0]))
                            else:
                                order = [(h, e) for e in pend for h in range(4)]
                            mm_done = set()
                            for h, (jt, pos, nD, ptD, ptP, relu01, mte, qe) in order:
                                if relu01 and jt not in mm_done:
                                    # v2-part of heads 0/1 via mask-matmul
                                    mm_done.add(jt)
                                    for k in range(2):
                                        nc.tensor.matmul(
                                            pgA[:, k * 512:(k + 1) * 512],
                                            g2pP[:, jt, :],
                                            mte[:, qe, k * 512:(k + 1) * 512],
                                            start=False, stop=False)
                                if h < nD:
                                    pair_mm(h, jt, ptD, h, pos)
                                else:
                                    pair_mm(h, jt, ptP, h - nD, pos)
                            pend = []

                # finalize
                ftp_ctx = tc.tile_pool(name="ftp", bufs=2, space="PSUM")
                ftp = ftp_ctx.__enter__()
                ndp_ctx = tc.tile_pool(name="ndp", bufs=4)
                ndp = ndp_ctx.__enter__()
                try:
                    for h in range(4):
                        tile_, off = pgt[h]
                        numD = ndp.tile([33, IR], f16, tag="numD")
                        if FINND[h] == "A":
                            nc.scalar.copy(numD[:], tile_[off:off + 33, :])
                        else:
                            nc.vector.tensor_copy(numD[:], tile_[off:off + 33, :])
                        tpA = ftp.tile([128, 8, 33], f32, tag="tpA")
                        for c in range(8):
                            nc.tensor.matmul(tpA[:, c, :],
                                             numD[:, c * 128:(c + 1) * 128],
                                             id33[:], start=True, stop=False)
                            nc.tensor.matmul(tpA[:, c, :],
                                             numD[32:33, c * 128:(c + 1) * 128],
                                             biasTE[32:33, h, :],
                                             start=False, stop=True)
                        rdT = ndp.tile([128, 8, 1], f32, tag="rdT")
                        nc.vector.reciprocal_approx_fast(rdT[:], tpA[:, :, 32:33])
                        nc.vector.tensor_tensor(
                            out_sb[:, :, h * 32:(h + 1) * 32], tpA[:, :, 0:32],
                            rdT[:, :, 0:1].broadcast_to([128, 8, 32]), op=Alu.mult)
                    nc.sync.dma_start(
                        out_d[:].rearrange("(s p) f -> p s f", p=128), out_sb[:])
                finally:
                    ndp_ctx.__exit__(None, None, None)
                    ftp_ctx.__exit__(None, None, None)
                    psg_ctx.__exit__(None, None, None)

            for _rep in range(reps):
                emit_body()
            pp_ctx.__exit__(None, None, None)
        finally:
            cst_ctx.__exit__(None, None, None)

    nc.compile()
    return nc


def _prepare_in_maps(x, adj, W, a_src, a_dst, bias):
    x = np.ascontiguousarray(np.asarray(x, dtype=np.float32))
    adj = np.asarray(adj, dtype=np.float32)
    W = np.asarray(W, dtype=np.float32)
    a_src = np.asarray(a_src, dtype=np.float32)
    a_dst = np.asarray(a_dst, dtype=np.float32)
    bias = np.asarray(bias, dtype=np.float32)

    Wf = np.ascontiguousarray(W.reshape(D, HF)) * 0.0625
    WaS = np.ascontiguousarray(np.einsum("dhf,hf->dh", W.reshape(D, H, F), a_src))
    WaD = np.ascontiguousarray(np.einsum("dhf,hf->dh", W.reshape(D, H, F), a_dst))
    biasRh = np.ascontiguousarray(bias.reshape(1, HF))

    in_maps = []
    for c in range(NCORES):
        b, cc = c // 2, c % 2
        i0 = cc * IR
        in_maps.append({
            "xT": np.ascontiguousarray(x[b].T),
            "xiT": np.ascontiguousarray(x[b, i0:i0 + IR].T),
            "adjT": np.ascontiguousarray(adj[b, i0:i0 + IR, :].T),
            "Wfs": Wf,
            "WaS": WaS,
            "WaD": WaD,
            "biasR": biasRh,
        })
    return in_maps


def run(inputs, trace=False, trace_cores=None):
    from concourse.bass_utils import run_bass_kernel_spmd
    if "nc" not in _CACHE:
        _CACHE["nc"] = build_nc()
    nc = _CACHE["nc"]
    in_maps = _prepare_in_maps(**inputs)
    kw = {}
    if trace:
        kw = dict(trace=True, trace_cores=trace_cores or [0])
    res = run_bass_kernel_spmd(nc, in_maps, list(range(NCORES)), **kw)
    out = np.zeros((B, N, HF), np.float32)
    for c in range(NCORES):
        b, cc = c // 2, c % 2
        out[b, cc * IR:(cc + 1) * IR, :] = res.results[c]["out"]
    return out, res


def kernel(**inputs):
    out, _ = run(inputs, trace=False)
    return out


# revision 87
# speedup vs baseline: 1.4453x; 1.0185x over previous
"""Batched GAT kernel for 8 Trainium2 NeuronCores.

Math: out[b,i,:] = softmax_j(mask(leakyrelu(s_i+t_j))) @ h  per head, concat heads.

Decomposition (per head): exp(lrelu(e)) / exp(.2 s_i) = max(w_i v1_j, v2_j)
with w = exp(.8 s), v1 = exp(t), v2 = exp(.2 t); the 1/exp(.2 s_i) row scale
cancels in the softmax ratio.

Layout: adj is HOST-transposed per core (adjT[j, i]), so the binarized mask is
born in the [j-partition, i-free] orientation the attention tiles use — no PE
transposes, no mask copies.  Per j-tile:
  q_h = tensor_scalar(wb_h, *v1_jh, max v2_jh)   (DVE, 4x mode)
  pt  = q * m  — split between one multi-head broadcast TT on DVE (2x mode)
        and a GPSIMD TT for 0-2 heads per tile (POOLH load-balance knob)
  pair matmuls vs Vpack = 0.0625*[h|1] accumulate num/den in PSUM, emitted in
  16-matmul per-block clusters to keep the PE p-state ramped.
s/t scores come from host-folded WaS = W@a_src, WaD = W@a_dst ([D,H]); the x
feature stream is processed in 512-column chunks interleaved with the adj
stream so first-tile compute starts ~6us in.  Finalize: transpose via 33x33
identity matmuls, reciprocal_approx_fast, fused bias matmul, single 512B-row
output DMA.
"""
import os
import sys
import numpy as np

for _p in ("/opt/trn_rl_repo",):
    if _p not in sys.path:
        sys.path.insert(0, _p)

B, N, D, H, F = 4, 2048, 128, 4, 32
HF = H * F           # 128
IR = 1024            # i-rows per core
NJT = N // 128       # 16 j-tiles
NCORES = 8

import json as _json

def _knob(name, default):
    v = os.environ.get(name)
    return _json.loads(v) if v else default

# adj blocks are processed in this order; the accumulation group opens on the
# first entry's j-tiles and closes on the last (process order is free since
# the PSUM accumulate is a sum)
BLK_ORDER = _knob("K_BORD", [7, 0, 1, 2, 3, 4, 5, 6])
# pool-head count per position in BLK_ORDER (load balance DVE vs GPSIMD;
# last positions 0 so GPSIMD drains before the finale)
POOLH_SEQ = _knob("K_PSEQ", [2, 1, 1, 2, 1, 2, 1, 1, 2, 1, 1, 1, 1, 1, 1, 1])
# blocks (by position) whose mask is binarized on GPSIMD (idle during
# startup) instead of waiting for the Act sigmoid table; DVE_BIN likewise on
# the vector engine (fills its pre-steady-state stall)
POOL_BIN = set(_knob("K_PBIN", [0]))
DVE_BIN = set(_knob("K_DBIN", []))
# positions where heads 0/1 use the Act-engine relu route:
# p~ = v2 + relu(v1*w - v2); the relu part goes through the normal masked
# pair matmuls and the v2 part is recovered by a mask-matmul against the
# v2-scaled feature pack (same PSUM accumulation group)
RELU_POS = set(_knob("K_RELU", [6, 7, 8, 9, 10, 11, 12, 13]))
FINND = _knob("K_FINND", "AAAA")
BUFS = _knob("K_BUFS", {})
BUFS = {"adjp": 4, "mtp": 5, "qdp": 3, "qpp": 4, "ptdp": 4, "ptpp": 4, **BUFS}

_CACHE = {}


def build_nc(reps=1):
    import concourse.bacc as bacc
    import concourse.tile as tile
    from concourse import mybir

    f32, f16 = mybir.dt.float32, mybir.dt.float16
    Alu = mybir.AluOpType
    Act = mybir.ActivationFunctionType

    nc = bacc.Bacc(None, target_bir_lowering=False)

    xT_d   = nc.dram_tensor("xT",   [D, N],    f32, kind="ExternalInput")
    xiT_d  = nc.dram_tensor("xiT",  [D, IR],   f32, kind="ExternalInput")
    adjT_d = nc.dram_tensor("adjT", [N, IR],   f32, kind="ExternalInput")
    Wf_d   = nc.dram_tensor("Wfs",  [D, HF],   f32, kind="ExternalInput")
    WaS_d  = nc.dram_tensor("WaS",  [D, H],    f32, kind="ExternalInput")
    WaD_d  = nc.dram_tensor("WaD",  [D, H],    f32, kind="ExternalInput")
    bias_d = nc.dram_tensor("biasR", [1, HF],  f32, kind="ExternalInput")
    out_d  = nc.dram_tensor("out",  [IR, HF],  f32, kind="ExternalOutput")

    EY = np.zeros((4, 4 * 128), np.float16)
    for h in range(H):
        EY[h, h * 128:(h + 1) * 128] = 1.0
    EY_d = nc.inline_tensor(EY, "EYc")
    ID33_d = nc.inline_tensor(np.eye(33, dtype=np.float16), "id33c")

    adjT_r = adjT_d[:].rearrange("(s p) i -> p s i", p=128)

    with tile.TileContext(nc) as tc:
        cst_ctx = tc.tile_pool(name="cst", bufs=1)
        cst = cst_ctx.__enter__()
        try:
            xiT   = cst.tile([D, IR], f32)
            xT    = cst.tile([D, N], f32)
            Wf    = cst.tile([D, HF], f32)
            WaS   = cst.tile([D, H], f32)
            WaD   = cst.tile([D, H], f32)
            biasR = cst.tile([1, HF], f32)
            biasTE = cst.tile([64, 4, 33], f16)
            eyc   = cst.tile([4, 4 * 128], f16)
            id33  = cst.tile([33, 33], f16)
            sigB  = cst.tile([128, 1], f32)

            Wf16  = cst.tile([D, HF], f16)
            WaS16 = cst.tile([D, H], f16)
            WaD16 = cst.tile([D, H], f16)
            xT16  = cst.tile([D, N], f16)
            xiT16 = cst.tile([D, IR], f16)
            sZ4   = cst.tile([4, IR], f32)
            w16   = cst.tile([4, IR], f16)          # exp(.8 s)
            wb16  = cst.tile([128, H, IR], f16)     # per-head broadcast of w16
            tAll  = cst.tile([128, NJT, H], f32)
            tv1   = cst.tile([128, NJT, H], f32)    # exp(t)
            tv2   = cst.tile([128, NJT, H], f32)    # exp(.2 t)
            tv2n  = cst.tile([128, NJT, H], f32)    # -exp(.2 t)
            Vpack = cst.tile([128, NJT, H, 33], f16)
            g2pP  = cst.tile([128, NJT, 97], f16)   # v2-scaled packs, pg rows
            out_sb = cst.tile([128, 8, HF], f32)

            # -------- DMA order: score path first, adj stream next --------
            nc.sync.dma_start(xiT[:], xiT_d[:])
            nc.sync.dma_start(WaS[:], WaS_d[:])
            # (adjT chunk DMAs are issued inside the block loop; pool bufs=3
            # lets the first three queue right behind these)
            nc.sync.dma_start(Wf[:], Wf_d[:])
            nc.sync.dma_start(WaD[:], WaD_d[:])
            nc.sync.dma_start(eyc[:], EY_d[:])
            nc.vector.memset(biasTE[:], 0.0)
            nc.gpsimd.memset(Vpack[:, :, :, 32:33], 0.0625)

            # -------- score/weight prep (DVE does the copies: it is idle
            # during startup and the bottleneck later) --------
            nc.vector.tensor_copy(WaS16[:], WaS[:])
            nc.vector.tensor_copy(WaD16[:], WaD[:])
            nc.vector.tensor_copy(Wf16[:], Wf[:])
            for k in range(2):
                nc.vector.tensor_copy(xiT16[:, k * 512:(k + 1) * 512],
                                      xiT[:, k * 512:(k + 1) * 512])

            pp_ctx = tc.tile_pool(name="pp", bufs=2, space="PSUM")
            pp = pp_ctx.__enter__()
            for k in range(2):
                ps = pp.tile([4, 512], f32, tag="pp")
                nc.tensor.matmul(ps[:], WaS16[:], xiT16[:, k * 512:(k + 1) * 512],
                                 start=True, stop=True)
                nc.vector.tensor_copy(sZ4[:, k * 512:(k + 1) * 512], ps[:])
            nc.scalar.activation(w16[:], sZ4[:], Act.Exp, scale=0.8)
            # w broadcasts immediately (DVE is idle now; h3 first since the
            # pool-side TS ops consume it soonest)
            for h in (3, 2, 1, 0):
                for k in range(2):
                    ps = pp.tile([128, 512], f32, tag="pp")
                    nc.tensor.matmul(ps[:], eyc[:, h * 128:(h + 1) * 128],
                                     w16[:, k * 512:(k + 1) * 512],
                                     start=True, stop=True)
                    nc.vector.tensor_copy(wb16[:, h, k * 512:(k + 1) * 512],
                                          ps[:])

            def xchunk_scores(g):
                # x columns [512g, 512(g+1)): t-scores + exps (gates sigmoids)
                nc.sync.dma_start(xT[:, g * 512:(g + 1) * 512],
                                  xT_d[:, g * 512:(g + 1) * 512])
                nc.scalar.copy(xT16[:, g * 512:(g + 1) * 512],
                               xT[:, g * 512:(g + 1) * 512])
                ps = pp.tile([128, 4 * H], f32, tag="pp")
                for k4 in range(4):
                    jt = g * 4 + k4
                    nc.tensor.matmul(ps[:, k4 * H:(k4 + 1) * H],
                                     xT16[:, jt * 128:(jt + 1) * 128], WaD16[:],
                                     start=True, stop=True)
                nc.scalar.copy(tAll[:, g * 4:(g + 1) * 4, :], ps[:])
                nc.scalar.activation(
                    tv1[:, g * 4:(g + 1) * 4, :].rearrange("p a b -> p (a b)"),
                    tAll[:, g * 4:(g + 1) * 4, :].rearrange("p a b -> p (a b)"),
                    Act.Exp)
                nc.scalar.activation(
                    tv2[:, g * 4:(g + 1) * 4, :].rearrange("p a b -> p (a b)"),
                    tAll[:, g * 4:(g + 1) * 4, :].rearrange("p a b -> p (a b)"),
                    Act.Exp, scale=0.2)

            def xchunk_vpack(blk):
                jt0 = 2 * blk
                pv = pp.tile([128, 256], f32, tag="pp")
                for d in range(2):
                    nc.tensor.matmul(
                        pv[:, d * 128:(d + 1) * 128],
                        xT16[:, (jt0 + d) * 128:(jt0 + d + 1) * 128],
                        Wf16[:], start=True, stop=True)
                nc.scalar.copy(
                    Vpack[:, jt0:jt0 + 2, :, 0:32],
                    pv[:].rearrange("p (j h f) -> p j h f", j=2, h=H))

            # late, fin-only params
            ID33_dma = lambda: (nc.sync.dma_start(id33[:], ID33_d[:]),
                                nc.sync.dma_start(biasR[:], bias_d[:]),
                                nc.scalar.copy(
                                    biasTE[32:33, :, 0:32],
                                    biasR[:].rearrange("p (h f) -> p h f", h=H)))

            def gate_sigmoid():
                # sigB is the sigmoid bias; producing it from the last exp
                # output (a Copy, so no table change) forces every Exp before
                # the first Sigmoid -> exactly one exp->sigmoid table switch.
                # g=3 first: BLK_ORDER starts at block 7 whose t-scores need
                # the last x column chunk.
                for g in (3, 0, 1, 2):
                    xchunk_scores(g)
                nc.scalar.activation(sigB[:], tv2[:, 11, 3:4], Act.Copy,
                                     bias=-5e5, scale=0.0)
                nc.vector.tensor_scalar(
                    tv2n[:].rearrange("p a b -> p (a b)"),
                    tv2[:].rearrange("p a b -> p (a b)"), -1.0, None,
                    op0=Alu.mult)
                if RELU_POS:
                    nc.vector.memset(g2pP[:, :, 33:64], 0.0)
                for blk in BLK_ORDER:
                    xchunk_vpack(blk)

            # ---------------- main body (per rep) ----------------
            def emit_body():
                psg_ctx = tc.tile_pool(name="psg", bufs=1, space="PSUM")
                psg = psg_ctx.__enter__()
                pgA = psg.tile([97, IR], f32, tag="pgA", name="pgA")
                pgB = psg.tile([97, IR], f32, tag="pgB", name="pgB")
                pgt = {0: (pgA, 0), 1: (pgA, 64), 2: (pgB, 0), 3: (pgB, 64)}

                def pair_mm(h, jt, pt, hh, pos):
                    tile_, off = pgt[h]
                    for k in range(2):
                        nc.tensor.matmul(
                            tile_[off:off + 33, k * 512:(k + 1) * 512],
                            Vpack[:, jt, h, :],
                            pt[:, hh, k * 512:(k + 1) * 512],
                            start=(pos == 0), stop=(pos == NJT - 1))

                with tc.tile_pool(name="adjp", bufs=BUFS["adjp"]) as adjp, \
                     tc.tile_pool(name="mtp", bufs=BUFS["mtp"]) as mtp, \
                     tc.tile_pool(name="qdp", bufs=BUFS["qdp"]) as qdp, \
                     tc.tile_pool(name="qpp", bufs=BUFS["qpp"]) as qpp, \
                     tc.tile_pool(name="ptdp", bufs=BUFS["ptdp"]) as ptdp, \
                     tc.tile_pool(name="ptpp", bufs=BUFS["ptpp"]) as ptpp:
                    # first adj chunks + their masks run pre-gate:
                    # DMA early, binarize on the (idle) GPSIMD
                    early = {}
                    for p_ in sorted(p for p in POOL_BIN if p < 2):
                        blk = BLK_ORDER[p_]
                        at = adjp.tile([128, 2, IR], f32, tag="adj")
                        nc.sync.dma_start(at[:], adjT_r[:, 2 * blk:2 * blk + 2, :])
                        mt = mtp.tile([128, 2, IR], f16, tag="mi")
                        nc.gpsimd.tensor_scalar(
                            mt[:], at[:], 0.5, None, op0=Alu.is_gt)
                        early[blk] = (at, mt)
                    gate_sigmoid()
                    pend = []
                    for bpos, blk in enumerate(BLK_ORDER):
                        if blk in early:
                            at, mt = early[blk]
                        else:
                            at = adjp.tile([128, 2, IR], f32, tag="adj")
                            nc.sync.dma_start(at[:],
                                              adjT_r[:, 2 * blk:2 * blk + 2, :])
                            mt = mtp.tile([128, 2, IR], f16, tag="mi")
                            if bpos in DVE_BIN:
                                nc.vector.tensor_scalar(
                                    mt[:], at[:], 0.5, None, op0=Alu.is_gt)
                            elif bpos in POOL_BIN:
                                nc.gpsimd.tensor_scalar(
                                    mt[:], at[:], 0.5, None, op0=Alu.is_gt)
                            else:
                                nc.scalar.activation(
                                    mt[:].rearrange("p a b -> p (a b)"),
                                    at[:].rearrange("p a b -> p (a b)"),
                                    Act.Sigmoid, bias=sigB[:, 0:1], scale=1e6)
                        for q in range(2):
                            jt = 2 * blk + q
                            pos = 2 * bpos + q
                            nP = POOLH_SEQ[pos]
                            nD = 4 - nP
                            # pool-side first so GPSIMD starts early
                            if nP:
                                qP = qpp.tile([128, 2, IR], f16, tag="qP")
                                for h in range(nD, 4):
                                    nc.vector.tensor_scalar(
                                        qP[:, h - nD, :], wb16[:, h, :],
                                        tv1[:, jt, h:h + 1], tv2[:, jt, h:h + 1],
                                        op0=Alu.mult, op1=Alu.max)
                                ptP = ptpp.tile([128, 2, IR], f16, tag="ptP")
                                nc.gpsimd.tensor_tensor(
                                    ptP[:, 0:nP, :], qP[:, 0:nP, :],
                                    mt[:, q:q + 1, :].broadcast_to([128, nP, IR]),
                                    op=Alu.mult)
                            else:
                                ptP = None
                            relu01 = pos in RELU_POS
                            if relu01:
                                for h in (0, 1):
                                    nc.vector.tensor_scalar(
                                        g2pP[:, jt, 64 * h:64 * h + 33],
                                        Vpack[:, jt, h, :],
                                        tv2[:, jt, h:h + 1], None, op0=Alu.mult)
                            qD = qdp.tile([128, 4, IR], f16, tag="qD")
                            for h in range(nD):
                                if relu01 and h < 2:
                                    nc.scalar.activation(
                                        qD[:, h, :], wb16[:, h, :], Act.Relu,
                                        scale=tv1[:, jt, h:h + 1],
                                        bias=tv2n[:, jt, h:h + 1])
                                else:
                                    nc.vector.tensor_scalar(
                                        qD[:, h, :], wb16[:, h, :],
                                        tv1[:, jt, h:h + 1], tv2[:, jt, h:h + 1],
                                        op0=Alu.mult, op1=Alu.max)
                            ptD = ptdp.tile([128, 4, IR], f16, tag="ptD")
                            nc.vector.tensor_tensor(
                                ptD[:, 0:nD, :], qD[:, 0:nD, :],
                                mt[:, q:q + 1, :].broadcast_to([128, nD, IR]),
                                op=Alu.mult)
                            pend.append((jt, pos, nD, ptD, ptP, relu01, mt, q))
                        if bpos == 6:
                            ID33_dma()
                        # 32-matmul PE clusters (every 2 blocks) keep the PE
                        # p-state ramped; the last block goes alone, DVE-fed
                        # matmuls first and h0/h1 before h2/h3, so the
                        # finalization overlaps the trailing mask work
                        if bpos % 2 == 1 or bpos >= 6:
                            if bpos == 7:
                                order = sorted(
                                    ((h, e) for h in range(4) for e in pend),
                                    key=lambda he: (he[0] >= 4 - he[1][2], he[0]))
                            else:
                                order = [(h, e) for e in pend for h in range(4)]
                            mm_done = set()
                            for h, (jt, pos, nD, ptD, ptP, relu01, mte, qe) in order:
                                if relu01 and jt not in mm_done:
                                    # v2-part of heads 0/1 via mask-matmul
                                    mm_done.add(jt)
                                    for k in range(2):
                                        nc.tensor.matmul(
                                            pgA[:, k * 512:(k + 1) * 512],
                                            g2pP[:, jt, :],
                                            mte[:, qe, k * 512:(k + 1) * 512],
                                            start=False, stop=False)
                                if h < nD:
                                    pair_mm(h, jt, ptD, h, pos)
                                else:
                                    pair_mm(h, jt, ptP, h - nD, pos)
                            pend = []

                # finalize
                ftp_ctx = tc.tile_pool(name="ftp", bufs=2, space="PSUM")
                ftp = ftp_ctx.__enter__()
                ndp_ctx = tc.tile_pool(name="ndp", bufs=4)
                ndp = ndp_ctx.__enter__()
                try:
                    for h in range(4):
                        tile_, off = pgt[h]
                        numD = ndp.tile([33, IR], f16, tag="numD")
                        if FINND[h] == "A":
                            nc.scalar.copy(numD[:], tile_[off:off + 33, :])
                        else:
                            nc.vector.tensor_copy(numD[:], tile_[off:off + 33, :])
                        tpA = ftp.tile([128, 8, 33], f32, tag="tpA")
                        for c in range(8):
                            nc.tensor.matmul(tpA[:, c, :],
                                             numD[:, c * 128:(c + 1) * 128],
                                             id33[:], start=True, stop=False)
                            nc.tensor.matmul(tpA[:, c, :],
                                             numD[32:33, c * 128:(c + 1) * 128],
                                             biasTE[32:33, h, :],
                                             start=False, stop=True)
                        rdT = ndp.tile([128, 8, 1], f32, tag="rdT")
                        nc.vector.reciprocal_approx_fast(rdT[:], tpA[:, :, 32:33])
                        nc.vector.tensor_tensor(
                            out_sb[:, :, h * 32:(h + 1) * 32], tpA[:, :, 0:32],
                            rdT[:, :, 0:1].broadcast_to([128, 8, 32]), op=Alu.mult)
                    nc.sync.dma_start(
                        out_d[:].rearrange("(s p) f -> p s f", p=128), out_sb[:])
                finally:
                    ndp_ctx.__exit__(None, None, None)
                    ftp_ctx.__exit__(None, None, None)
                    psg_ctx.__exit__(None, None, None)

            for _rep in range(reps):
                emit_body()
            pp_ctx.__exit__(None, None, None)
        finally:
            cst_ctx.__exit__(None, None, None)

    nc.compile()
    return nc


def _prepare_in_maps(x, adj, W, a_src, a_dst, bias):
    x = np.ascontiguousarray(np.asarray(x, dtype=np.float32))
    adj = np.asarray(adj, dtype=np.float32)
    W = np.asarray(W, dtype=np.float32)
    a_src = np.asarray(a_src, dtype=np.float32)
    a_dst = np.asarray(a_dst, dtype=np.float32)
    bias = np.asarray(bias, dtype=np.float32)

    Wf = np.ascontiguousarray(W.reshape(D, HF)) * 0.0625
    WaS = np.ascontiguousarray(np.einsum("dhf,hf->dh", W.reshape(D, H, F), a_src))
    WaD = np.ascontiguousarray(np.einsum("dhf,hf->dh", W.reshape(D, H, F), a_dst))
    biasRh = np.ascontiguousarray(bias.reshape(1, HF))

    in_maps = []
    for c in range(NCORES):
        b, cc = c // 2, c % 2
        i0 = cc * IR
        in_maps.append({
            "xT": np.ascontiguousarray(x[b].T),
            "xiT": np.ascontiguousarray(x[b, i0:i0 + IR].T),
            "adjT": np.ascontiguousarray(adj[b, i0:i0 + IR, :].T),
            "Wfs": Wf,
            "WaS": WaS,
            "WaD": WaD,
            "biasR": biasRh,
        })
    return in_maps


def run(inputs, trace=False, trace_cores=None):
    from concourse.bass_utils import run_bass_kernel_spmd
    if "nc" not in _CACHE:
        _CACHE["nc"] = build_nc()
    nc = _CACHE["nc"]
    in_maps = _prepare_in_maps(**inputs)
    kw = {}
    if trace:
        kw = dict(trace=True, trace_cores=trace_cores or [0])
    res = run_bass_kernel_spmd(nc, in_maps, list(range(NCORES)), **kw)
    out = np.zeros((B, N, HF), np.float32)
    for c in range(NCORES):
        b, cc = c // 2, c % 2
        out[b, cc * IR:(cc + 1) * IR, :] = res.results[c]["out"]
    return out, res


def kernel(**inputs):
    out, _ = run(inputs, trace=False)
    return out
